# revision 10
# baseline (speedup 1.0000x reference)
"""Trainium2 Bass kernel for nn_BaselineModel (sampling + MSResNet + FC head).

Contract: kernel(**inputs) takes FULL unsharded inputs (x [32,100,30,1000] f32,
padding_mask [32,100,30] bool, params pytree) and returns the FULL output
[32, 2] f32.  Internally: batch is sharded 4-per-core across 8 NeuronCores;
the electrode/trial sampling indices (which depend only on padding_mask and a
fixed PRNG key) are computed on host, the selected rows are gathered and laid
out as matmul-ready images, and the whole MSResNet + head runs on-device in
fp32r matmuls.
"""

import os
import numpy as np

B, E, TR, T = 32, 100, 30, 1000
N_ELECS, EMBED, NCLS = 25, 128, 2
NCORES, BC = 8, 4
L1, L2, L3 = 500, 250, 125
KS = (3, 5, 7)

LAST_RESULTS = None
_PROG = None


# ---------------------------------------------------------------- host side


def _sample_indices(padding_mask):
    """Bit-exact replication of the reference's electrode/trial sampling."""
    import jax

    # IMPORTANT: no device/impl overrides here — must match the ambient code
    # path reference.py uses, which yields different streams than e.g.
    # running under jax.default_device(cpu).
    k1, k2 = jax.random.split(jax.random.key(42))
    eg = np.asarray(jax.random.uniform(k1, (B, E)), np.float32)
    tg = np.asarray(jax.random.uniform(k2, (B, N_ELECS, TR)), np.float32)
    vt_full = ~padding_mask
    valid_elec = vt_full.any(-1)
    scores = np.where(valid_elec, eg, np.float32(-1.0))
    sel = np.argsort(-scores, axis=-1, kind="stable")[:, :N_ELECS]
    vt = np.take_along_axis(vt_full, sel[:, :, None], axis=1)
    trial = np.argmax(np.where(vt, tg, np.float32(-1.0)), axis=-1)
    return sel, trial


class _Pack:
    """Packs [rows<=128, cols] f32 regions into a [128, W] image; regions with
    rows<=64 are paired top/bottom to halve DMA bytes."""

    def __init__(self):
        self.cols = 0
        self.regions = {}
        self.pending = []
        self.arrays = []

    def add(self, name, arr):
        arr = np.ascontiguousarray(arr, np.float32)
        r, c = arr.shape
        assert r <= 128
        # no top/bottom pairing: matmul requires lhsT.base_partition ==
        # rhs.base_partition, and all conv inputs live at base 0
        c0 = self.cols
        self.cols += c
        self._place(name, 0, c0, arr)

    def _place(self, name, r0, c0, arr):
        self.regions[name] = (r0, c0, arr.shape[0], arr.shape[1])
        self.arrays.append((r0, c0, arr))

    def image(self):
        img = np.zeros((128, max(self.cols, 1)), np.float32)
        for r0, c0, a in self.arrays:
            img[r0 : r0 + a.shape[0], c0 : c0 + a.shape[1]] = a
        return img


def _conv_lhst(w):
    """[cout, cin, k] -> [cin, k*cout] with col j*cout+co = w[co, ci, j]."""
    cout, cin, k = w.shape
    return np.ascontiguousarray(np.transpose(w, (1, 2, 0)).reshape(cin, k * cout))


def _pack_weights(params):
    p = {k: np.asarray(v, np.float32) for k, v in _flatten(params).items()}

    packs = {"w0": _Pack()}
    w0 = packs["w0"]
    fc = np.concatenate(
        [p["fc_w"][i * 128 : (i + 1) * 128] / 125.0 for i in range(3)], axis=1
    )
    w0.add("fc", fc)
    w0.add("cl1", p["cl1_w"])
    stem = np.transpose(p["stem_w"], (2, 1, 0))  # [7, 25, 64]
    w0.add("stemA", stem[:5].reshape(125, 64))
    w0.add("id64", np.eye(64, dtype=np.float32))
    w0.add("stemB", stem[5:7].reshape(50, 64))
    w0.add("cl2", p["cl2_w"])

    for i, k in enumerate(KS):
        pk = _Pack()
        packs[f"w{k}"] = pk
        pk.add("b2c2", _conv_lhst(p[f"br{i}.b2.w2"]))
        pk.add("b2c1", _conv_lhst(p[f"br{i}.b2.w1"]))
        pk.add("down", _conv_lhst(p[f"br{i}.b2.wd"]))
        pk.add("b1c1", _conv_lhst(p[f"br{i}.b1.w1"]))
        pk.add("b1c2", _conv_lhst(p[f"br{i}.b1.w2"]))

    bimg = np.zeros((128, 16), np.float32)
    bimg[:64, 0] = p["stem_b"]
    for i in range(3):
        bimg[:64, 1 + 4 * i] = p[f"br{i}.b1.b1"]
        bimg[:64, 2 + 4 * i] = p[f"br{i}.b1.b2"]
        bimg[:, 3 + 4 * i] = p[f"br{i}.b2.b1"]
        bimg[:, 4 + 4 * i] = p[f"br{i}.b2.b2"] + p[f"br{i}.b2.bd"]
    bimg[:, 13] = p["fc_b"]
    bimg[:, 14] = p["cl1_b"]
    bimg[:NCLS, 15] = p["cl2_b"]

    images = {name: pk.image() for name, pk in packs.items()}
    regions = {name: pk.regions for name, pk in packs.items()}
    return images, regions, bimg


def _flatten(params):
    out = {}
    for k, v in params.items():
        if k == "branches":
            for i, br in enumerate(v):
                for bn, bd in br.items():
                    for wn, wv in bd.items():
                        out[f"br{i}.{bn}.{wn}"] = wv
        else:
            out[k] = v
    return out


def _stem_images(xg):
    """xg: [BC, 25, 1000] gathered rows for one core.  Returns xa [125, 2000]
    (taps 0-4) and xb [50, 2000] (taps 5-6), with column order
    (chunk, b, t_local), chunk = 125 output positions."""
    xp = np.zeros((BC, N_ELECS, T + 6), np.float32)
    xp[:, :, 3 : 3 + T] = xg
    xa = np.empty((125, BC * L1), np.float32)
    xb = np.empty((50, BC * L1), np.float32)
    for j in range(7):
        arr = xp[:, :, j : j + 2 * L1 : 2]  # [BC, 25, 500]
        # -> [25, chunk, b, tl] -> [25, 2000]
        a4 = arr.transpose(1, 0, 2).reshape(N_ELECS, BC, 4, 125)
        # column order (chunk, t_local, b): batch-interleaved inner dim
        a4 = np.ascontiguousarray(a4.transpose(0, 2, 3, 1)).reshape(N_ELECS, BC * L1)
        if j < 5:
            xa[j * 25 : (j + 1) * 25] = a4
        else:
            xb[(j - 5) * 25 : (j - 4) * 25] = a4
    return xa, xb


# -------------------------------------------------------------- device side


def _build_program(wcols):
    import concourse.mybir as mybir
    from concourse import bacc
    from concourse.tile import TileContext

    F32, F32R = mybir.dt.float32, mybir.dt.float32r
    AF = mybir.ActivationFunctionType
    ALU = mybir.AluOpType

    nc = bacc.Bacc("TRN2", target_bir_lowering=False, num_devices=NCORES)

    d_xa = nc.dram_tensor("xa", [125, BC * L1], F32R, kind="ExternalInput")
    d_xb = nc.dram_tensor("xb", [50, BC * L1], F32R, kind="ExternalInput")
    d_bs = nc.dram_tensor("bs", [128, 16], F32, kind="ExternalInput")
    d_w = {
        name: nc.dram_tensor(name, [128, wcols[name]], F32R, kind="ExternalInput")
        for name in ("w0", "w3", "w5", "w7")
    }
    d_y = nc.dram_tensor("y", [NCLS, BC], F32, kind="ExternalOutput")

    with TileContext(nc) as tc:
        with (
            tc.tile_pool(name="const", bufs=1) as cpool,
            tc.tile_pool(name="act", bufs=1) as apool,
            tc.tile_pool(name="work", bufs=2) as wpool,
            tc.tile_pool(name="pp", bufs=6, space="PSUM") as ppool,
            tc.tile_pool(name="ph", bufs=2, space="PSUM") as phpool,
        ):
            # ---- input DMAs: chunked and spread round-robin across engine
            # queues, in consumption-priority order
            dma_engines = [nc.sync, nc.gpsimd, nc.scalar]
            _ei = [0]

            def dma(out, in_):
                dma_engines[_ei[0] % len(dma_engines)].dma_start(out=out, in_=in_)
                _ei[0] += 1

            xa_t = cpool.tile([125, BC * L1], F32R, tag="xa")
            xb_t = cpool.tile([50, BC * L1], F32R, tag="xb")
            bs_t = cpool.tile([128, 16], F32, tag="bs")
            w_t = {}
            for img in ("w0", "w3", "w5", "w7"):
                w_t[img] = cpool.tile([128, wcols[img]], F32R, tag=img,
                                      name=f"{img}_t")
            # priority: stem inputs + w0 first, then branch weights by k
            for c in range(4):
                cs = slice(c * 500, (c + 1) * 500)
                dma(xa_t[:, cs], d_xa[:, cs])
                dma(xb_t[:, cs], d_xb[:, cs])
            dma(bs_t[:], d_bs[:])
            for img in ("w0", "w3", "w5", "w7"):
                W = wcols[img]
                nch = max(1, round(W / 1200))
                bounds = [W * i // nch for i in range(nch + 1)]
                for a, b in zip(bounds, bounds[1:]):
                    dma(w_t[img][:, a:b], d_w[img][:, a:b])

            def wreg(img, rname, cin, j, cout):
                r0, c0, rr, cc = _REGIONS[img][rname]
                return w_t[img][0:cin, c0 + j * cout : c0 + (j + 1) * cout]

            def bias(col, rows=128):
                return bs_t[0:rows, col : col + 1]

            # ---- persistent activation tiles, batch-interleaved layout:
            # flat column index = 4*t_padded + b
            h1 = apool.tile([64, BC * (L1 + 2)], F32R, tag="h1")
            h2 = apool.tile([64, BC * (L2 + 6)], F32R, tag="h2")
            y1 = {k: apool.tile([64, BC * (L2 + 6)], F32R, tag=f"y1_{k}", name=f"y1_{k}") for k in KS}
            y2 = {k: apool.tile([64, BC * (L2 + 6)], F32R, tag=f"y2_{k}", name=f"y2_{k}") for k in KS}
            y2e = {k: apool.tile([64, BC * 128], F32R, tag=f"y2e_{k}", name=f"y2e_{k}") for k in KS}
            y2o = {k: apool.tile([64, BC * 128], F32R, tag=f"y2o_{k}", name=f"y2o_{k}") for k in KS}
            z1 = {k: apool.tile([128, BC * (L3 + 6)], F32R, tag=f"z1_{k}", name=f"z1_{k}") for k in KS}
            feats = apool.tile([128, 12], F32R, tag="feats")

            def pad_memset(tile, parts, lp, interior_l, lpad):
                u32 = mybir.dt.uint32
                if lpad:
                    nc.vector.memset(tile[0:parts, 0 : BC * lpad].bitcast(u32), 0)
                rpad_start = lpad + interior_l
                if rpad_start < lp:
                    nc.vector.memset(
                        tile[0:parts, BC * rpad_start : BC * lp].bitcast(u32), 0)

            pad_memset(h1, 64, L1 + 2, L1, 1)
            pad_memset(h2, 64, L2 + 6, L2, 3)
            for k in KS:
                pad_memset(y1[k], 64, L2 + 6, L2, 3)
                pad_memset(y2[k], 64, L2 + 6, L2, 3)
                pad_memset(z1[k], 128, L3 + 6, L3, 3)

            # ---- stem: 4 chunks of 125 output positions
            for c in range(4):
                pt = ppool.tile([128, 500], F32, tag="pt")
                cs = slice(c * 500, (c + 1) * 500)
                nc.tensor.matmul(
                    pt[0:64, :], lhsT=wreg("w0", "stemA", 125, 0, 64),
                    rhs=xa_t[:, cs], start=True, stop=False,
                )
                nc.tensor.matmul(
                    pt[0:64, :], lhsT=wreg("w0", "stemB", 50, 0, 64),
                    rhs=xb_t[:, cs], start=False, stop=True,
                )
                nc.scalar.activation(
                    h1[0:64, BC * (1 + 125 * c) : BC * (126 + 125 * c)],
                    pt[0:64, :], AF.Relu, bias=bias(0, 64),
                )

            # ---- maxpool 3 s2 p1: out t reads h1 padded idx {2t, 2t+1, 2t+2}
            # h1 flat idx = 4*tp + b -> window w: [64, (t: stride 8), (b: 4)]
            h1w = h1.rearrange("c (t b) -> c t b", b=BC)
            pm = wpool.tile([64, BC * 250], F32R, tag="pm")
            for half in range(2):
                t0 = 125 * half
                a = h1w[0:64, 2 * t0 + 0 : 2 * t0 + 250 : 2, :]
                bb = h1w[0:64, 2 * t0 + 1 : 2 * t0 + 251 : 2, :]
                cc = h1w[0:64, 2 * t0 + 2 : 2 * t0 + 252 : 2, :]
                pmv = pm[:, BC * t0 : BC * (t0 + 125)].rearrange(
                    "c (t b) -> c t b", b=BC)
                h2v = h2[0:64, BC * (3 + t0) : BC * (3 + t0 + 125)].rearrange(
                    "c (t b) -> c t b", b=BC)
                nc.vector.tensor_tensor(pmv, a, bb, ALU.max)
                nc.vector.tensor_tensor(h2v, pmv, cc, ALU.max)

            # ---- branches
            def conv(img, rname, in_t, cin, cout, K, Lout, out_t=None,
                     out_off=0, bias_col=None, epilogue=None, extra=None,
                     in_odd=None, stride=1):
                """tap-accumulated conv over batch-interleaved tiles.
                stride==1: taps are contiguous slices of in_t.
                stride==2: in_t is the even-parity tile, in_odd the odd one."""
                p = (K - 1) // 2
                nch = (BC * Lout + 499) // 500
                cn = Lout // nch
                for c in range(nch):
                    pt = ppool.tile([128, 500], F32, tag="pt")
                    po = pt[0:cout, 0 : BC * cn]
                    for j in range(K):
                        o = (3 - p) + j
                        if stride == 1:
                            rhs = in_t[0:cin, BC * (o + c * cn) : BC * (o + (c + 1) * cn)]
                        else:
                            src, oo = (in_t, o // 2) if o % 2 == 0 else (in_odd, o // 2)
                            rhs = src[0:cin, BC * (oo + c * cn) : BC * (oo + (c + 1) * cn)]
                        nc.tensor.matmul(
                            po, lhsT=wreg(img, rname, cin, j, cout), rhs=rhs,
                            start=(j == 0), stop=(j == K - 1 and extra is None),
                        )
                    if extra is not None:
                        extra(po, c, cn)
                    if epilogue == "act":
                        nc.scalar.activation(
                            out_t[0:cout, BC * (out_off + c * cn) : BC * (out_off + (c + 1) * cn)],
                            po, AF.Relu, bias=bias(bias_col, cout),
                        )
                    else:
                        epilogue(pt, c, cn)

            for i, k in enumerate(KS):
                wk = f"w{k}"
                # b1 conv1: 64->64, stride 1, relu
                conv(wk, "b1c1", h2, 64, 64, k, L2,
                     out_t=y1[k], out_off=3, bias_col=1 + 4 * i, epilogue="act")

                # b1 conv2 + identity shortcut, relu
                def sc_extra(po, c, cn):
                    nc.tensor.matmul(
                        po, lhsT=wreg("w0", "id64", 64, 0, 64),
                        rhs=h2[0:64, BC * (3 + c * cn) : BC * (3 + (c + 1) * cn)],
                        start=False, stop=True,
                    )
                conv(wk, "b1c2", y1[k], 64, 64, k, L2,
                     out_t=y2[k], out_off=3, bias_col=2 + 4 * i, epilogue="act",
                     extra=sc_extra)

                # parity-split copies of y2 for the stride-2 convs
                y2w = y2[k].rearrange("c (t b) -> c t b", b=BC)
                nc.vector.tensor_copy(
                    out=y2e[k][:, :].rearrange("c (t b) -> c t b", b=BC),
                    in_=y2w[0:64, 0:256:2, :])
                nc.vector.tensor_copy(
                    out=y2o[k][:, :].rearrange("c (t b) -> c t b", b=BC),
                    in_=y2w[0:64, 1:256:2, :])

                # b2 conv1: 64->128, stride 2, relu
                conv(wk, "b2c1", y2e[k], 64, 128, k, L3,
                     out_t=z1[k], out_off=3, bias_col=3 + 4 * i, epilogue="act",
                     in_odd=y2o[k], stride=2)

                # b2 conv2 + downsample; relu+bias then time-sum into feats
                def ds_extra(po, c, cn):
                    nc.tensor.matmul(
                        po, lhsT=wreg(wk, "down", 64, 0, 128),
                        rhs=y2o[k][0:64, BC * 1 : BC * 126],
                        start=False, stop=True,
                    )

                def pool_epilogue(pt, c, cn, _i=i):
                    rt = wpool.tile([128, BC * L3], F32R, tag="rt")
                    nc.vector.tensor_scalar(
                        rt[:], pt[0:128, 0 : BC * L3], bias(4 + 4 * _i), 0.0,
                        ALU.add, ALU.max,
                    )
                    with nc.allow_low_precision(reason="f32r feature sums"):
                        nc.vector.tensor_reduce(
                            feats[:, _i * 4 : (_i + 1) * 4],
                            rt.rearrange("c (t b) -> c b t", b=BC),
                            mybir.AxisListType.X, ALU.add,
                        )
                conv(wk, "b2c2", z1[k], 128, 128, k, L3,
                     epilogue=pool_epilogue, extra=ds_extra)

            # ---- head
            pe = phpool.tile([128, 8], F32, tag="ph")
            for br in range(3):
                nc.tensor.matmul(
                    pe[:, 0:4], lhsT=wreg("w0", "fc", 128, br, 128),
                    rhs=feats[:, br * 4 : (br + 1) * 4],
                    start=(br == 0), stop=(br == 2),
                )
            embt = wpool.tile([128, 4], F32R, tag="embt")
            nc.scalar.activation(embt[:], pe[:, 0:4], AF.Identity, bias=bias(13))

            ph2 = phpool.tile([128, 8], F32, tag="ph")
            nc.tensor.matmul(ph2[:, 0:4], lhsT=wreg("w0", "cl1", 128, 0, 128),
                             rhs=embt[:], start=True, stop=True)
            hct = wpool.tile([128, 4], F32R, tag="hct")
            nc.scalar.activation(hct[:], ph2[:, 0:4], AF.Relu, bias=bias(14))

            ph3 = phpool.tile([128, 8], F32, tag="ph")
            nc.tensor.matmul(ph3[0:NCLS, 0:4], lhsT=wreg("w0", "cl2", 128, 0, NCLS),
                             rhs=hct[:], start=True, stop=True)
            outt = wpool.tile([NCLS, 4], F32, tag="outt")
            nc.scalar.activation(outt[:], ph3[0:NCLS, 0:4], AF.Identity,
                                 bias=bias(15, NCLS))
            nc.sync.dma_start(out=d_y[:], in_=outt[:])

    nc.compile()
    return nc


_REGIONS = None


def _prepare(x, padding_mask, params):
    global _REGIONS
    x = np.asarray(x, np.float32)
    padding_mask = np.asarray(padding_mask, bool)
    sel, trial = _sample_indices(padding_mask)
    images, regions, bimg = _pack_weights(params)
    _REGIONS = regions

    in_maps = []
    for c in range(NCORES):
        bs = slice(c * BC, (c + 1) * BC)
        xs = x[bs]
        xg = xs[np.arange(BC)[:, None], sel[bs], trial[bs]]  # [BC, 25, 1000]
        xa, xb = _stem_images(xg)
        im = {"xa": xa, "xb": xb, "bs": bimg}
        im.update(images)
        in_maps.append(im)
    wcols = {name: arr.shape[1] for name, arr in images.items()}
    return in_maps, wcols


def kernel(x, padding_mask, params):
    global _PROG, LAST_RESULTS
    from concourse.bass_utils import run_bass_kernel_spmd

    in_maps, wcols = _prepare(x, padding_mask, params)
    if _PROG is None:
        _PROG = _build_program(wcols)

    trace = os.environ.get("BASS_KERNEL_TRACE", "0") == "1"
    res = run_bass_kernel_spmd(
        _PROG, in_maps, core_ids=list(range(NCORES)), trace=trace
    )
    LAST_RESULTS = res
    out = np.empty((B, NCLS), np.float32)
    for c in range(NCORES):
        out[c * BC : (c + 1) * BC] = res.results[c]["y"].T
    return out


# revision 11
# speedup vs baseline: 1.2131x; 1.2131x over previous
"""Trainium2 Bass kernel for nn_BaselineModel (sampling + MSResNet + FC head).

Contract: kernel(**inputs) takes FULL unsharded inputs (x [32,100,30,1000] f32,
padding_mask [32,100,30] bool, params pytree) and returns the FULL output
[32, 2] f32.  Internally: batch is sharded 4-per-core across 8 NeuronCores;
the electrode/trial sampling indices (which depend only on padding_mask and a
fixed PRNG key) are computed on host, the selected rows are gathered and laid
out as matmul-ready images, and the whole MSResNet + head runs on-device in
fp32r matmuls.
"""

import os
import numpy as np

B, E, TR, T = 32, 100, 30, 1000
N_ELECS, EMBED, NCLS = 25, 128, 2
NCORES, BC = 8, 4
L1, L2, L3 = 500, 250, 125
KS = (3, 5, 7)

LAST_RESULTS = None
_PROG = None
COMPUTE = os.environ.get("BASS_KERNEL_DT", "bf16")  # "bf16" | "f32r"


def _to_compute(a):
    if COMPUTE == "bf16":
        import ml_dtypes

        return np.ascontiguousarray(a).astype(ml_dtypes.bfloat16)
    return np.ascontiguousarray(a, np.float32)


# ---------------------------------------------------------------- host side


def _sample_indices(padding_mask):
    """Bit-exact replication of the reference's electrode/trial sampling."""
    import jax

    # IMPORTANT: no device/impl overrides here — must match the ambient code
    # path reference.py uses, which yields different streams than e.g.
    # running under jax.default_device(cpu).
    k1, k2 = jax.random.split(jax.random.key(42))
    eg = np.asarray(jax.random.uniform(k1, (B, E)), np.float32)
    tg = np.asarray(jax.random.uniform(k2, (B, N_ELECS, TR)), np.float32)
    vt_full = ~padding_mask
    valid_elec = vt_full.any(-1)
    scores = np.where(valid_elec, eg, np.float32(-1.0))
    sel = np.argsort(-scores, axis=-1, kind="stable")[:, :N_ELECS]
    vt = np.take_along_axis(vt_full, sel[:, :, None], axis=1)
    trial = np.argmax(np.where(vt, tg, np.float32(-1.0)), axis=-1)
    return sel, trial


class _Pack:
    """Packs [rows<=128, cols] f32 regions into a [128, W] image; regions with
    rows<=64 are paired top/bottom to halve DMA bytes."""

    def __init__(self):
        self.cols = 0
        self.regions = {}
        self.pending = []
        self.arrays = []

    def add(self, name, arr):
        arr = np.ascontiguousarray(arr, np.float32)
        r, c = arr.shape
        assert r <= 128
        # no top/bottom pairing: matmul requires lhsT.base_partition ==
        # rhs.base_partition, and all conv inputs live at base 0
        c0 = self.cols
        self.cols += c
        self._place(name, 0, c0, arr)

    def _place(self, name, r0, c0, arr):
        self.regions[name] = (r0, c0, arr.shape[0], arr.shape[1])
        self.arrays.append((r0, c0, arr))

    def image(self):
        img = np.zeros((128, max(self.cols, 1)), np.float32)
        for r0, c0, a in self.arrays:
            img[r0 : r0 + a.shape[0], c0 : c0 + a.shape[1]] = a
        return img


def _conv_lhst(w):
    """[cout, cin, k] -> [cin, k*cout] with col j*cout+co = w[co, ci, j]."""
    cout, cin, k = w.shape
    return np.ascontiguousarray(np.transpose(w, (1, 2, 0)).reshape(cin, k * cout))


def _pack_weights(params):
    p = {k: np.asarray(v, np.float32) for k, v in _flatten(params).items()}

    packs = {"w0": _Pack()}
    w0 = packs["w0"]
    stem = np.transpose(p["stem_w"], (2, 1, 0))  # [7, 25, 64]
    w0.add("stemA", stem[:5].reshape(125, 64))
    w0.add("stemB", stem[5:7].reshape(50, 64))
    w0.add("id64", np.eye(64, dtype=np.float32))
    fc = np.concatenate(
        [p["fc_w"][i * 128 : (i + 1) * 128] / 125.0 for i in range(3)], axis=1
    )
    w0.add("fc", fc)
    w0.add("cl1", p["cl1_w"])
    w0.add("cl2", p["cl2_w"])

    for i, k in enumerate(KS):
        pk = _Pack()
        packs[f"w{k}"] = pk
        pk.add("b2c2", _conv_lhst(p[f"br{i}.b2.w2"]))
        pk.add("b2c1", _conv_lhst(p[f"br{i}.b2.w1"]))
        pk.add("down", _conv_lhst(p[f"br{i}.b2.wd"]))
        pk.add("b1c1", _conv_lhst(p[f"br{i}.b1.w1"]))
        pk.add("b1c2", _conv_lhst(p[f"br{i}.b1.w2"]))

    bimg = np.zeros((128, 16), np.float32)
    bimg[:64, 0] = p["stem_b"]
    for i in range(3):
        bimg[:64, 1 + 4 * i] = p[f"br{i}.b1.b1"]
        bimg[:64, 2 + 4 * i] = p[f"br{i}.b1.b2"]
        bimg[:, 3 + 4 * i] = p[f"br{i}.b2.b1"]
        bimg[:, 4 + 4 * i] = p[f"br{i}.b2.b2"] + p[f"br{i}.b2.bd"]
    bimg[:, 13] = p["fc_b"]
    bimg[:, 14] = p["cl1_b"]
    bimg[:NCLS, 15] = p["cl2_b"]

    images = {name: pk.image() for name, pk in packs.items()}
    regions = {name: pk.regions for name, pk in packs.items()}
    return images, regions, bimg


def _flatten(params):
    out = {}
    for k, v in params.items():
        if k == "branches":
            for i, br in enumerate(v):
                for bn, bd in br.items():
                    for wn, wv in bd.items():
                        out[f"br{i}.{bn}.{wn}"] = wv
        else:
            out[k] = v
    return out


def _stem_images(xg):
    """xg: [BC, 25, 1000] gathered rows for one core.  Returns xa [125, 2000]
    (taps 0-4) and xb [50, 2000] (taps 5-6), with column order
    (chunk, b, t_local), chunk = 125 output positions."""
    xp = np.zeros((BC, N_ELECS, T + 6), np.float32)
    xp[:, :, 3 : 3 + T] = xg
    xa = np.empty((125, BC * L1), np.float32)
    xb = np.empty((50, BC * L1), np.float32)
    for j in range(7):
        arr = xp[:, :, j : j + 2 * L1 : 2]  # [BC, 25, 500]
        # -> [25, chunk, b, tl] -> [25, 2000]
        a4 = arr.transpose(1, 0, 2).reshape(N_ELECS, BC, 4, 125)
        # column order (chunk, t_local, b): batch-interleaved inner dim
        a4 = np.ascontiguousarray(a4.transpose(0, 2, 3, 1)).reshape(N_ELECS, BC * L1)
        if j < 5:
            xa[j * 25 : (j + 1) * 25] = a4
        else:
            xb[(j - 5) * 25 : (j - 4) * 25] = a4
    return xa, xb


# -------------------------------------------------------------- device side


def _build_program(wcols):
    import concourse.mybir as mybir
    from concourse import bacc
    from concourse.tile import TileContext

    F32 = mybir.dt.float32
    F32R = mybir.dt.bfloat16 if COMPUTE == "bf16" else mybir.dt.float32r
    AF = mybir.ActivationFunctionType
    ALU = mybir.AluOpType

    nc = bacc.Bacc("TRN2", target_bir_lowering=False, num_devices=NCORES)

    d_xa = nc.dram_tensor("xa", [125, BC * L1], F32R, kind="ExternalInput")
    d_xb = nc.dram_tensor("xb", [50, BC * L1], F32R, kind="ExternalInput")
    d_bs = nc.dram_tensor("bs", [128, 16], F32, kind="ExternalInput")
    d_w = {
        name: nc.dram_tensor(name, [128, wcols[name]], F32R, kind="ExternalInput")
        for name in ("w0", "w3", "w5", "w7")
    }
    d_y = nc.dram_tensor("y", [NCLS, BC], F32, kind="ExternalOutput")

    with TileContext(nc) as tc:
        with (
            tc.tile_pool(name="const", bufs=1) as cpool,
            tc.tile_pool(name="act", bufs=1) as apool,
            tc.tile_pool(name="work", bufs=2) as wpool,
            tc.tile_pool(name="pp", bufs=6, space="PSUM") as ppool,
            tc.tile_pool(name="ph", bufs=2, space="PSUM") as phpool,
        ):
            # ---- input DMAs: chunked and spread round-robin across engine
            # queues, in consumption-priority order
            dma_engines = [nc.sync, nc.scalar]
            _ei = [0]

            def dma(out, in_):
                dma_engines[_ei[0] % len(dma_engines)].dma_start(out=out, in_=in_)
                _ei[0] += 1

            xa_t = cpool.tile([125, BC * L1], F32R, tag="xa")
            xb_t = cpool.tile([50, BC * L1], F32R, tag="xb")
            bs_t = cpool.tile([128, 16], F32, tag="bs")
            w_t = {}
            for img in ("w0", "w3", "w5", "w7"):
                w_t[img] = cpool.tile([128, wcols[img]], F32R, tag=img,
                                      name=f"{img}_t")
            # priority order: stem weights + first stem chunk, then the rest
            stem_end = _REGIONS["w0"]["id64"][1]
            dma(w_t["w0"][:, 0:stem_end], d_w["w0"][:, 0:stem_end])
            for c in range(2):
                cs = slice(c * 1000, (c + 1) * 1000)
                dma(xa_t[:, cs], d_xa[:, cs])
                dma(xb_t[:, cs], d_xb[:, cs])
            dma(bs_t[:], d_bs[:])
            dma(w_t["w0"][:, stem_end:], d_w["w0"][:, stem_end:])
            for img in ("w3", "w5", "w7"):
                dma(w_t[img][:], d_w[img][:])

            def wreg(img, rname, cin, j, cout):
                r0, c0, rr, cc = _REGIONS[img][rname]
                return w_t[img][0:cin, c0 + j * cout : c0 + (j + 1) * cout]

            def bias(col, rows=128):
                return bs_t[0:rows, col : col + 1]

            # ---- persistent activation tiles, batch-interleaved layout:
            # flat column index = 4*t_padded + b
            h1 = apool.tile([64, BC * (L1 + 2)], F32R, tag="h1")
            h2 = apool.tile([64, BC * (L2 + 6)], F32R, tag="h2")
            y1 = {k: apool.tile([64, BC * (L2 + 6)], F32R, tag=f"y1_{k}", name=f"y1_{k}") for k in KS}
            y2 = {k: apool.tile([64, BC * (L2 + 6)], F32R, tag=f"y2_{k}", name=f"y2_{k}") for k in KS}
            y2e = {k: apool.tile([64, BC * 128], F32R, tag=f"y2e_{k}", name=f"y2e_{k}") for k in KS}
            y2o = {k: apool.tile([64, BC * 128], F32R, tag=f"y2o_{k}", name=f"y2o_{k}") for k in KS}
            z1 = {k: apool.tile([128, BC * (L3 + 6)], F32R, tag=f"z1_{k}", name=f"z1_{k}") for k in KS}
            feats32 = apool.tile([128, 12], F32, tag="feats32")
            feats = apool.tile([128, 12], F32R, tag="feats")

            def pad_memset(tile, parts, lp, interior_l, lpad):
                u32 = mybir.dt.uint32
                if lpad:
                    nc.vector.memset(tile[0:parts, 0 : BC * lpad].bitcast(u32), 0)
                rpad_start = lpad + interior_l
                if rpad_start < lp:
                    nc.vector.memset(
                        tile[0:parts, BC * rpad_start : BC * lp].bitcast(u32), 0)

            pad_memset(h1, 64, L1 + 2, L1, 1)
            pad_memset(h2, 64, L2 + 6, L2, 3)
            for k in KS:
                pad_memset(y1[k], 64, L2 + 6, L2, 3)
                pad_memset(y2[k], 64, L2 + 6, L2, 3)
                pad_memset(z1[k], 128, L3 + 6, L3, 3)

            # ---- stem: 4 chunks of 125 output positions
            for c in range(4):
                pt = ppool.tile([128, 500], F32, tag="pt")
                cs = slice(c * 500, (c + 1) * 500)
                nc.tensor.matmul(
                    pt[0:64, :], lhsT=wreg("w0", "stemA", 125, 0, 64),
                    rhs=xa_t[:, cs], start=True, stop=False,
                )
                nc.tensor.matmul(
                    pt[0:64, :], lhsT=wreg("w0", "stemB", 50, 0, 64),
                    rhs=xb_t[:, cs], start=False, stop=True,
                )
                nc.scalar.activation(
                    h1[0:64, BC * (1 + 125 * c) : BC * (126 + 125 * c)],
                    pt[0:64, :], AF.Relu, bias=bias(0, 64),
                )

            # ---- maxpool 3 s2 p1: out t reads h1 padded idx {2t, 2t+1, 2t+2}
            # h1 flat idx = 4*tp + b -> window w: [64, (t: stride 8), (b: 4)]
            h1w = h1.rearrange("c (t b) -> c t b", b=BC)
            pm = wpool.tile([64, BC * 250], F32R, tag="pm")
            for half in range(2):
                t0 = 125 * half
                a = h1w[0:64, 2 * t0 + 0 : 2 * t0 + 250 : 2, :]
                bb = h1w[0:64, 2 * t0 + 1 : 2 * t0 + 251 : 2, :]
                cc = h1w[0:64, 2 * t0 + 2 : 2 * t0 + 252 : 2, :]
                pmv = pm[:, BC * t0 : BC * (t0 + 125)].rearrange(
                    "c (t b) -> c t b", b=BC)
                h2v = h2[0:64, BC * (3 + t0) : BC * (3 + t0 + 125)].rearrange(
                    "c (t b) -> c t b", b=BC)
                nc.vector.tensor_tensor(pmv, a, bb, ALU.max)
                nc.vector.tensor_tensor(h2v, pmv, cc, ALU.max)

            # ---- branches
            def conv(img, rname, in_t, cin, cout, K, Lout, out_t=None,
                     out_off=0, bias_col=None, epilogue=None, extra=None,
                     in_odd=None, stride=1):
                """tap-accumulated conv over batch-interleaved tiles.
                stride==1: taps are contiguous slices of in_t.
                stride==2: in_t is the even-parity tile, in_odd the odd one."""
                p = (K - 1) // 2
                nch = (BC * Lout + 499) // 500
                cn = Lout // nch
                for c in range(nch):
                    pt = ppool.tile([128, 500], F32, tag="pt")
                    po = pt[0:cout, 0 : BC * cn]
                    for j in range(K):
                        o = (3 - p) + j
                        if stride == 1:
                            rhs = in_t[0:cin, BC * (o + c * cn) : BC * (o + (c + 1) * cn)]
                        else:
                            src, oo = (in_t, o // 2) if o % 2 == 0 else (in_odd, o // 2)
                            rhs = src[0:cin, BC * (oo + c * cn) : BC * (oo + (c + 1) * cn)]
                        nc.tensor.matmul(
                            po, lhsT=wreg(img, rname, cin, j, cout), rhs=rhs,
                            start=(j == 0), stop=(j == K - 1 and extra is None),
                        )
                    if extra is not None:
                        extra(po, c, cn)
                    if epilogue == "act":
                        nc.scalar.activation(
                            out_t[0:cout, BC * (out_off + c * cn) : BC * (out_off + (c + 1) * cn)],
                            po, AF.Relu, bias=bias(bias_col, cout),
                        )
                    else:
                        epilogue(pt, c, cn)

            for i, k in enumerate(KS):
                wk = f"w{k}"
                # b1 conv1: 64->64, stride 1, relu
                conv(wk, "b1c1", h2, 64, 64, k, L2,
                     out_t=y1[k], out_off=3, bias_col=1 + 4 * i, epilogue="act")

                # b1 conv2 + identity shortcut, relu
                def sc_extra(po, c, cn):
                    nc.tensor.matmul(
                        po, lhsT=wreg("w0", "id64", 64, 0, 64),
                        rhs=h2[0:64, BC * (3 + c * cn) : BC * (3 + (c + 1) * cn)],
                        start=False, stop=True,
                    )
                conv(wk, "b1c2", y1[k], 64, 64, k, L2,
                     out_t=y2[k], out_off=3, bias_col=2 + 4 * i, epilogue="act",
                     extra=sc_extra)

                # parity-split copies of y2 for the stride-2 convs
                y2w = y2[k].rearrange("c (t b) -> c t b", b=BC)
                nc.vector.tensor_copy(
                    out=y2e[k][:, :].rearrange("c (t b) -> c t b", b=BC),
                    in_=y2w[0:64, 0:256:2, :])
                nc.vector.tensor_copy(
                    out=y2o[k][:, :].rearrange("c (t b) -> c t b", b=BC),
                    in_=y2w[0:64, 1:256:2, :])

                # b2 conv1: 64->128, stride 2, relu
                conv(wk, "b2c1", y2e[k], 64, 128, k, L3,
                     out_t=z1[k], out_off=3, bias_col=3 + 4 * i, epilogue="act",
                     in_odd=y2o[k], stride=2)

                # b2 conv2 + downsample; relu+bias then time-sum into feats
                def ds_extra(po, c, cn):
                    nc.tensor.matmul(
                        po, lhsT=wreg(wk, "down", 64, 0, 128),
                        rhs=y2o[k][0:64, BC * 1 : BC * 126],
                        start=False, stop=True,
                    )

                def pool_epilogue(pt, c, cn, _i=i):
                    rt = wpool.tile([128, BC * L3], F32R, tag="rt")
                    nc.vector.tensor_scalar(
                        rt[:], pt[0:128, 0 : BC * L3], bias(4 + 4 * _i), 0.0,
                        ALU.add, ALU.max,
                    )
                    nc.vector.tensor_reduce(
                        feats32[:, _i * 4 : (_i + 1) * 4],
                        rt.rearrange("c (t b) -> c b t", b=BC),
                        mybir.AxisListType.X, ALU.add,
                    )
                    nc.vector.tensor_copy(
                        out=feats[:, _i * 4 : (_i + 1) * 4],
                        in_=feats32[:, _i * 4 : (_i + 1) * 4],
                    )
                conv(wk, "b2c2", z1[k], 128, 128, k, L3,
                     epilogue=pool_epilogue, extra=ds_extra)

            # ---- head
            pe = phpool.tile([128, 8], F32, tag="ph")
            for br in range(3):
                nc.tensor.matmul(
                    pe[:, 0:4], lhsT=wreg("w0", "fc", 128, br, 128),
                    rhs=feats[:, br * 4 : (br + 1) * 4],
                    start=(br == 0), stop=(br == 2),
                )
            embt = wpool.tile([128, 4], F32R, tag="embt")
            nc.scalar.activation(embt[:], pe[:, 0:4], AF.Identity, bias=bias(13))

            ph2 = phpool.tile([128, 8], F32, tag="ph")
            nc.tensor.matmul(ph2[:, 0:4], lhsT=wreg("w0", "cl1", 128, 0, 128),
                             rhs=embt[:], start=True, stop=True)
            hct = wpool.tile([128, 4], F32R, tag="hct")
            nc.scalar.activation(hct[:], ph2[:, 0:4], AF.Relu, bias=bias(14))

            ph3 = phpool.tile([128, 8], F32, tag="ph")
            nc.tensor.matmul(ph3[0:NCLS, 0:4], lhsT=wreg("w0", "cl2", 128, 0, NCLS),
                             rhs=hct[:], start=True, stop=True)
            outt = wpool.tile([NCLS, 4], F32, tag="outt")
            nc.scalar.activation(outt[:], ph3[0:NCLS, 0:4], AF.Identity,
                                 bias=bias(15, NCLS))
            nc.sync.dma_start(out=d_y[:], in_=outt[:])

    nc.compile()
    return nc


_REGIONS = None


def _prepare(x, padding_mask, params):
    global _REGIONS
    x = np.asarray(x, np.float32)
    padding_mask = np.asarray(padding_mask, bool)
    sel, trial = _sample_indices(padding_mask)
    images, regions, bimg = _pack_weights(params)
    cimages = {k: _to_compute(v) for k, v in images.items()}
    _REGIONS = regions

    in_maps = []
    for c in range(NCORES):
        bs = slice(c * BC, (c + 1) * BC)
        xs = x[bs]
        xg = xs[np.arange(BC)[:, None], sel[bs], trial[bs]]  # [BC, 25, 1000]
        xa, xb = _stem_images(xg)
        im = {"xa": _to_compute(xa), "xb": _to_compute(xb), "bs": bimg}
        im.update(cimages)
        in_maps.append(im)
    wcols = {name: arr.shape[1] for name, arr in images.items()}
    return in_maps, wcols


def kernel(x, padding_mask, params):
    global _PROG, LAST_RESULTS
    from concourse.bass_utils import run_bass_kernel_spmd

    in_maps, wcols = _prepare(x, padding_mask, params)
    if _PROG is None:
        _PROG = _build_program(wcols)

    trace = os.environ.get("BASS_KERNEL_TRACE", "0") == "1"
    res = run_bass_kernel_spmd(
        _PROG, in_maps, core_ids=list(range(NCORES)), trace=trace
    )
    LAST_RESULTS = res
    out = np.empty((B, NCLS), np.float32)
    for c in range(NCORES):
        out[c * BC : (c + 1) * BC] = res.results[c]["y"].T
    return out


# revision 13
# speedup vs baseline: 1.4475x; 1.1932x over previous
"""Trainium2 Bass kernel for nn_BaselineModel (sampling + MSResNet + FC head).

Contract: kernel(**inputs) takes FULL unsharded inputs (x [32,100,30,1000] f32,
padding_mask [32,100,30] bool, params pytree) and returns the FULL output
[32, 2] f32.  Internally: batch is sharded 4-per-core across 8 NeuronCores;
the electrode/trial sampling indices (which depend only on padding_mask and a
fixed PRNG key) are computed on host, the selected rows are gathered and laid
out as matmul-ready images, and the whole MSResNet + head runs on-device in
fp32r matmuls.
"""

import os
import numpy as np

B, E, TR, T = 32, 100, 30, 1000
N_ELECS, EMBED, NCLS = 25, 128, 2
NCORES, BC = 8, 4
L1, L2, L3 = 500, 250, 125
KS = (3, 5, 7)

LAST_RESULTS = None
_PROG = None
COMPUTE = os.environ.get("BASS_KERNEL_DT", "bf16")  # "bf16" | "f32r"


def _to_compute(a):
    if COMPUTE == "bf16":
        import ml_dtypes

        return np.ascontiguousarray(a).astype(ml_dtypes.bfloat16)
    return np.ascontiguousarray(a, np.float32)


# ---------------------------------------------------------------- host side


def _sample_indices(padding_mask):
    """Bit-exact replication of the reference's electrode/trial sampling."""
    import jax

    # IMPORTANT: no device/impl overrides here — must match the ambient code
    # path reference.py uses, which yields different streams than e.g.
    # running under jax.default_device(cpu).
    k1, k2 = jax.random.split(jax.random.key(42))
    eg = np.asarray(jax.random.uniform(k1, (B, E)), np.float32)
    tg = np.asarray(jax.random.uniform(k2, (B, N_ELECS, TR)), np.float32)
    vt_full = ~padding_mask
    valid_elec = vt_full.any(-1)
    scores = np.where(valid_elec, eg, np.float32(-1.0))
    sel = np.argsort(-scores, axis=-1, kind="stable")[:, :N_ELECS]
    vt = np.take_along_axis(vt_full, sel[:, :, None], axis=1)
    trial = np.argmax(np.where(vt, tg, np.float32(-1.0)), axis=-1)
    return sel, trial


class _Pack:
    """Packs [rows<=128, cols] f32 regions into a [128, W] image; regions with
    rows<=64 are paired top/bottom to halve DMA bytes."""

    def __init__(self):
        self.cols = 0
        self.regions = {}
        self.pending = []
        self.arrays = []

    def add(self, name, arr):
        arr = np.ascontiguousarray(arr, np.float32)
        r, c = arr.shape
        assert r <= 128
        # no top/bottom pairing: matmul requires lhsT.base_partition ==
        # rhs.base_partition, and all conv inputs live at base 0
        c0 = self.cols
        self.cols += c
        self._place(name, 0, c0, arr)

    def _place(self, name, r0, c0, arr):
        self.regions[name] = (r0, c0, arr.shape[0], arr.shape[1])
        self.arrays.append((r0, c0, arr))

    def image(self):
        img = np.zeros((128, max(self.cols, 1)), np.float32)
        for r0, c0, a in self.arrays:
            img[r0 : r0 + a.shape[0], c0 : c0 + a.shape[1]] = a
        return img


def _conv_lhst_tap(w, j):
    """[cout, cin, k] tap j -> lhsT [cin, cout]."""
    return np.ascontiguousarray(w[:, :, j].T)


def _groups(k):
    """Tap offsets o in [3-p, 3+p] grouped into even-start pairs + solos."""
    p = (k - 1) // 2
    o, out = 3 - p, []
    while o <= 3 + p:
        if o % 2 == 0 and o + 1 <= 3 + p:
            out.append((o, True))
            o += 2
        else:
            out.append((o, False))
            o += 1
    return out


def _pack_weights(params):
    p = {k: np.asarray(v, np.float32) for k, v in _flatten(params).items()}

    packs = {"w0": _Pack()}
    w0 = packs["w0"]
    stem = np.transpose(p["stem_w"], (2, 1, 0))  # [7, 25, 64]
    w0.add("stemA", stem[:5].reshape(125, 64))
    w0.add("stemB", stem[5:7].reshape(50, 64))
    w0.add("id64", np.eye(64, dtype=np.float32))
    fc = np.concatenate(
        [p["fc_w"][i * 128 : (i + 1) * 128] / 125.0 for i in range(3)], axis=1
    )
    w0.add("fc", fc)
    w0.add("cl1", p["cl1_w"])
    w0.add("cl2", p["cl2_w"])

    def paired(w, k):
        """Return (pairs [128, npair*cout], solos [64, nsolo*cout])."""
        cout = w.shape[0]
        pcv = (k - 1) // 2
        prs, sls = [], []
        for o, ispair in _groups(k):
            j = o - (3 - pcv)
            if ispair:
                blk = np.zeros((128, cout), np.float32)
                blk[0:64] = _conv_lhst_tap(w, j)
                blk[64:128] = _conv_lhst_tap(w, j + 1)
                prs.append(blk)
            else:
                sls.append(_conv_lhst_tap(w, j))
        pa = np.concatenate(prs, axis=1) if prs else np.zeros((128, 0), np.float32)
        sa = np.concatenate(sls, axis=1) if sls else np.zeros((64, 0), np.float32)
        return pa, sa

    for i, k in enumerate(KS):
        pk = _Pack()
        packs[f"w{k}"] = pk
        for cname, wkey in (("b1c1", f"br{i}.b1.w1"), ("b1c2", f"br{i}.b1.w2"),
                            ("b2c1", f"br{i}.b2.w1")):
            pa, sa = paired(p[wkey], k)
            if pa.shape[1]:
                pk.add(f"{cname}_p", pa)
            if sa.shape[1]:
                pk.add(f"{cname}_s", sa)
        # b2c2: full 128 contraction, one tap per group
        w2 = p[f"br{i}.b2.w2"]
        pk.add("b2c2", np.concatenate(
            [_conv_lhst_tap(w2, j) for j in range(k)], axis=1))
        pk.add("down", _conv_lhst_tap(p[f"br{i}.b2.wd"], 0))

    bimg = np.zeros((128, 16), np.float32)
    bimg[:64, 0] = p["stem_b"]
    for i in range(3):
        bimg[:64, 1 + 4 * i] = p[f"br{i}.b1.b1"]
        bimg[:64, 2 + 4 * i] = p[f"br{i}.b1.b2"]
        bimg[:, 3 + 4 * i] = p[f"br{i}.b2.b1"]
        bimg[:, 4 + 4 * i] = p[f"br{i}.b2.b2"] + p[f"br{i}.b2.bd"]
    bimg[:, 13] = p["fc_b"]
    bimg[:, 14] = p["cl1_b"]
    bimg[:NCLS, 15] = p["cl2_b"]

    images = {name: pk.image() for name, pk in packs.items()}
    regions = {name: pk.regions for name, pk in packs.items()}
    return images, regions, bimg


def _flatten(params):
    out = {}
    for k, v in params.items():
        if k == "branches":
            for i, br in enumerate(v):
                for bn, bd in br.items():
                    for wn, wv in bd.items():
                        out[f"br{i}.{bn}.{wn}"] = wv
        else:
            out[k] = v
    return out


def _stem_images(xg):
    """xg: [BC, 25, 1000] gathered rows for one core.  Returns xa [125, 2000]
    (taps 0-4) and xb [50, 2000] (taps 5-6), with column order
    (chunk, b, t_local), chunk = 125 output positions."""
    xp = np.zeros((BC, N_ELECS, T + 6), np.float32)
    xp[:, :, 3 : 3 + T] = xg
    xa = np.empty((125, BC * L1), np.float32)
    xb = np.empty((50, BC * L1), np.float32)
    for j in range(7):
        arr = xp[:, :, j : j + 2 * L1 : 2]  # [BC, 25, 500]
        # -> [25, chunk, b, tl] -> [25, 2000]
        a4 = arr.transpose(1, 0, 2).reshape(N_ELECS, BC, 4, 125)
        # column order (chunk, t_local, b): batch-interleaved inner dim
        a4 = np.ascontiguousarray(a4.transpose(0, 2, 3, 1)).reshape(N_ELECS, BC * L1)
        if j < 5:
            xa[j * 25 : (j + 1) * 25] = a4
        else:
            xb[(j - 5) * 25 : (j - 4) * 25] = a4
    return xa, xb


# -------------------------------------------------------------- device side


def _build_program(wcols):
    import concourse.mybir as mybir
    from concourse import bacc
    from concourse.tile import TileContext

    F32 = mybir.dt.float32
    F32R = mybir.dt.bfloat16 if COMPUTE == "bf16" else mybir.dt.float32r
    AF = mybir.ActivationFunctionType
    ALU = mybir.AluOpType

    nc = bacc.Bacc("TRN2", target_bir_lowering=False, num_devices=NCORES)

    d_xa = nc.dram_tensor("xa", [125, BC * L1], F32R, kind="ExternalInput")
    d_xb = nc.dram_tensor("xb", [50, BC * L1], F32R, kind="ExternalInput")
    d_bs = nc.dram_tensor("bs", [128, 16], F32, kind="ExternalInput")
    d_w = {
        name: nc.dram_tensor(name, [128, wcols[name]], F32R, kind="ExternalInput")
        for name in ("w0", "w3", "w5", "w7")
    }
    d_y = nc.dram_tensor("y", [NCLS, BC], F32, kind="ExternalOutput")

    with TileContext(nc) as tc:
        with (
            tc.tile_pool(name="const", bufs=1) as cpool,
            tc.tile_pool(name="act", bufs=1) as apool,
            tc.tile_pool(name="work", bufs=2) as wpool,
            tc.tile_pool(name="pp", bufs=6, space="PSUM") as ppool,
            tc.tile_pool(name="ph", bufs=2, space="PSUM") as phpool,
        ):
            dma_engines = [nc.sync, nc.scalar]
            _ei = [0]

            def dma(out, in_):
                dma_engines[_ei[0] % len(dma_engines)].dma_start(out=out, in_=in_)
                _ei[0] += 1

            # preload the ACT function table while DMAs run
            dummy = wpool.tile([1, 1], F32, tag="dummy")
            nc.vector.memset(dummy[:], 0.0)
            nc.scalar.activation(dummy[:], dummy[:], AF.Relu)

            xa_t = cpool.tile([125, BC * L1], F32R, tag="xa")
            xb_t = cpool.tile([50, BC * L1], F32R, tag="xb")
            bs_t = cpool.tile([128, 16], F32, tag="bs")
            w_t = {}
            for img in ("w0", "w3", "w5", "w7"):
                w_t[img] = cpool.tile([128, wcols[img]], F32R, tag=img,
                                      name=f"{img}_t")
            stem_end = _REGIONS["w0"]["id64"][1]
            dma(w_t["w0"][:, 0:stem_end], d_w["w0"][:, 0:stem_end])
            dma(xa_t[:], d_xa[:])
            dma(xb_t[:], d_xb[:])
            dma(bs_t[:], d_bs[:])
            dma(w_t["w3"][:], d_w["w3"][:])
            dma(w_t["w5"][:], d_w["w5"][:])
            dma(w_t["w7"][:], d_w["w7"][:])
            dma(w_t["w0"][:, stem_end:], d_w["w0"][:, stem_end:])

            def wreg(img, rname, rows, j, cout):
                r0, c0, rr, cc = _REGIONS[img][rname]
                return w_t[img][0:rows, c0 + j * cout : c0 + (j + 1) * cout]

            def bias(col, rows=128):
                return bs_t[0:rows, col : col + 1]

            # ---- persistent tiles, batch-interleaved (col = BC*t + b).
            # d-tiles: rows 0:64 = tensor, rows 64:128 = tensor shifted by one
            # t (filled by an SBUF->SBUF DMA), enabling 128-deep tap pairs.
            h1 = apool.tile([64, BC * (L1 + 2)], F32R, tag="h1")
            h2d = apool.tile([128, BC * (L2 + 6)], F32R, tag="h2d")
            y1d = {k: apool.tile([128, BC * (L2 + 6)], F32R, tag=f"y1d_{k}", name=f"y1d_{k}") for k in KS}
            y2d = {k: apool.tile([128, BC * (L2 + 6)], F32R, tag=f"y2d_{k}", name=f"y2d_{k}") for k in KS}
            z1 = {k: apool.tile([128, BC * (L3 + 6)], F32R, tag=f"z1_{k}", name=f"z1_{k}") for k in KS}
            feats32 = apool.tile([128, 12], F32, tag="feats32")
            feats = apool.tile([128, 12], F32R, tag="feats")

            def pad_memset(tile, parts, lp, interior_l, lpad):
                u32 = mybir.dt.uint32
                if lpad:
                    nc.vector.memset(tile[0:parts, 0 : BC * lpad].bitcast(u32), 0)
                rs = lpad + interior_l
                if rs < lp:
                    nc.vector.memset(tile[0:parts, BC * rs : BC * lp].bitcast(u32), 0)

            pad_memset(h1, 64, L1 + 2, L1, 1)
            pad_memset(h2d, 64, L2 + 6, L2, 3)
            for k in KS:
                pad_memset(y1d[k], 64, L2 + 6, L2, 3)
                pad_memset(y2d[k], 64, L2 + 6, L2, 3)
                pad_memset(z1[k], 128, L3 + 6, L3, 3)

            def shift_fill(dt_, lp):
                # bottom[t] = top[t+1]; last bottom column stays unread
                dma(dt_[64:128, 0 : BC * (lp - 1)], dt_[0:64, BC * 1 : BC * lp])

            # ---- stem
            for c in range(4):
                pt = ppool.tile([128, 500], F32, tag="pt")
                cs = slice(c * 500, (c + 1) * 500)
                nc.tensor.matmul(pt[0:64, :], lhsT=wreg("w0", "stemA", 125, 0, 64),
                                 rhs=xa_t[:, cs], start=True, stop=False)
                nc.tensor.matmul(pt[0:64, :], lhsT=wreg("w0", "stemB", 50, 0, 64),
                                 rhs=xb_t[:, cs], start=False, stop=True)
                nc.scalar.activation(
                    h1[0:64, BC * (1 + 125 * c) : BC * (126 + 125 * c)],
                    pt[0:64, :], AF.Relu, bias=bias(0, 64),
                )

            # ---- maxpool 3 s2 p1 into h2d top
            h1w = h1.rearrange("c (t b) -> c t b", b=BC)
            pm = wpool.tile([64, BC * 250], F32R, tag="pm")
            for half in range(2):
                t0 = 125 * half
                a = h1w[0:64, 2 * t0 + 0 : 2 * t0 + 250 : 2, :]
                bb = h1w[0:64, 2 * t0 + 1 : 2 * t0 + 251 : 2, :]
                cc = h1w[0:64, 2 * t0 + 2 : 2 * t0 + 252 : 2, :]
                pmv = pm[:, BC * t0 : BC * (t0 + 125)].rearrange(
                    "c (t b) -> c t b", b=BC)
                h2v = h2d[0:64, BC * (3 + t0) : BC * (3 + t0 + 125)].rearrange(
                    "c (t b) -> c t b", b=BC)
                nc.vector.tensor_tensor(pmv, a, bb, ALU.max)
                nc.vector.tensor_tensor(h2v, pmv, cc, ALU.max)
            shift_fill(h2d, L2 + 6)

            # ---- paired-tap conv over a d-tile (stride 1 contiguous slices)
            def conv_pair(img, cname, dt_, cout, k, Lout, out_t=None, out_off=0,
                          bias_col=None, epilogue=None, extra=None, stride=1):
                nch = (BC * Lout + 499) // 500
                cn = Lout // nch
                dt3 = dt_.rearrange("c (t b) -> c t b", b=BC)
                for c in range(nch):
                    pt = ppool.tile([128, 500], F32, tag="pt")
                    po = pt[0:cout, 0 : BC * cn]
                    gi_p = gi_s = 0
                    groups = _groups(k)
                    for gi, (o, ispair) in enumerate(groups):
                        lastg = gi == len(groups) - 1
                        if ispair:
                            lhsT = wreg(img, f"{cname}_p", 128, gi_p, cout)
                            gi_p += 1
                            rows = 128
                        else:
                            lhsT = wreg(img, f"{cname}_s", 64, gi_s, cout)
                            gi_s += 1
                            rows = 64
                        if stride == 1:
                            rhs = dt_[0:rows, BC * (o + c * cn) : BC * (o + c * cn + cn)]
                        else:
                            rhs = dt3[0:rows, o : o + 2 * cn : 2, :]
                        nc.tensor.matmul(po, lhsT=lhsT, rhs=rhs,
                                         start=(gi == 0),
                                         stop=(lastg and extra is None))
                    if extra is not None:
                        extra(po, c, cn)
                    if epilogue == "act":
                        nc.scalar.activation(
                            out_t[0:cout, BC * (out_off + c * cn) : BC * (out_off + (c + 1) * cn)],
                            po, AF.Relu, bias=bias(bias_col, cout),
                        )
                    else:
                        epilogue(pt, c, cn)

            # ---- wave-interleaved branch stages
            for i, k in enumerate(KS):
                conv_pair(f"w{k}", "b1c1", h2d, 64, k, L2,
                          out_t=y1d[k], out_off=3, bias_col=1 + 4 * i,
                          epilogue="act")
            for k in KS:
                shift_fill(y1d[k], L2 + 6)

            for i, k in enumerate(KS):
                def sc_extra(po, c, cn):
                    nc.tensor.matmul(
                        po, lhsT=wreg("w0", "id64", 64, 0, 64),
                        rhs=h2d[0:64, BC * (3 + c * cn) : BC * (3 + (c + 1) * cn)],
                        start=False, stop=True,
                    )
                conv_pair(f"w{k}", "b1c2", y1d[k], 64, k, L2,
                          out_t=y2d[k], out_off=3, bias_col=2 + 4 * i,
                          epilogue="act", extra=sc_extra)
            for k in KS:
                shift_fill(y2d[k], L2 + 6)

            for i, k in enumerate(KS):
                conv_pair(f"w{k}", "b2c1", y2d[k], 128, k, L3,
                          out_t=z1[k], out_off=3, bias_col=3 + 4 * i,
                          epilogue="act", stride=2)

            for i, k in enumerate(KS):
                wk = f"w{k}"
                y2d3 = y2d[k].rearrange("c (t b) -> c t b", b=BC)

                def ds_extra(po, c, cn, _k=k, _y=y2d3):
                    nc.tensor.matmul(
                        po, lhsT=wreg(f"w{_k}", "down", 64, 0, 128),
                        rhs=_y[0:64, 3 : 253 : 2, :],
                        start=False, stop=True,
                    )

                def pool_epilogue(pt, c, cn, _i=i):
                    rt = wpool.tile([128, BC * L3], F32R, tag="rt")
                    nc.vector.tensor_scalar(
                        rt[:], pt[0:128, 0 : BC * L3], bias(4 + 4 * _i), 0.0,
                        ALU.add, ALU.max,
                    )
                    nc.vector.tensor_reduce(
                        feats32[:, _i * 4 : (_i + 1) * 4],
                        rt.rearrange("c (t b) -> c b t", b=BC),
                        mybir.AxisListType.X, ALU.add,
                    )
                    nc.vector.tensor_copy(
                        out=feats[:, _i * 4 : (_i + 1) * 4],
                        in_=feats32[:, _i * 4 : (_i + 1) * 4],
                    )

                # b2c2: full-128 contraction, one tap per matmul
                p = (k - 1) // 2
                pt = ppool.tile([128, 500], F32, tag="pt")
                po = pt[0:128, 0 : BC * L3]
                for j in range(k):
                    o = (3 - p) + j
                    rhs = z1[k][0:128, BC * o : BC * (o + L3)]
                    nc.tensor.matmul(po, lhsT=wreg(wk, "b2c2", 128, j, 128),
                                     rhs=rhs, start=(j == 0), stop=False)
                ds_extra(po, 0, L3)
                pool_epilogue(pt, 0, L3)

            # ---- head
            pe = phpool.tile([128, 8], F32, tag="ph")
            for br in range(3):
                nc.tensor.matmul(
                    pe[:, 0:4], lhsT=wreg("w0", "fc", 128, br, 128),
                    rhs=feats[:, br * 4 : (br + 1) * 4],
                    start=(br == 0), stop=(br == 2),
                )
            embt = wpool.tile([128, 4], F32R, tag="embt")
            nc.scalar.activation(embt[:], pe[:, 0:4], AF.Identity, bias=bias(13))

            ph2 = phpool.tile([128, 8], F32, tag="ph")
            nc.tensor.matmul(ph2[:, 0:4], lhsT=wreg("w0", "cl1", 128, 0, 128),
                             rhs=embt[:], start=True, stop=True)
            hct = wpool.tile([128, 4], F32R, tag="hct")
            nc.scalar.activation(hct[:], ph2[:, 0:4], AF.Relu, bias=bias(14))

            ph3 = phpool.tile([128, 8], F32, tag="ph")
            nc.tensor.matmul(ph3[0:NCLS, 0:4], lhsT=wreg("w0", "cl2", 128, 0, NCLS),
                             rhs=hct[:], start=True, stop=True)
            outt = wpool.tile([NCLS, 4], F32, tag="outt")
            nc.scalar.activation(outt[:], ph3[0:NCLS, 0:4], AF.Identity,
                                 bias=bias(15, NCLS))
            nc.sync.dma_start(out=d_y[:], in_=outt[:])

    nc.compile()
    return nc


_REGIONS = None


def _prepare(x, padding_mask, params):
    global _REGIONS
    x = np.asarray(x, np.float32)
    padding_mask = np.asarray(padding_mask, bool)
    sel, trial = _sample_indices(padding_mask)
    images, regions, bimg = _pack_weights(params)
    cimages = {k: _to_compute(v) for k, v in images.items()}
    _REGIONS = regions

    in_maps = []
    for c in range(NCORES):
        bs = slice(c * BC, (c + 1) * BC)
        xs = x[bs]
        xg = xs[np.arange(BC)[:, None], sel[bs], trial[bs]]  # [BC, 25, 1000]
        xa, xb = _stem_images(xg)
        im = {"xa": _to_compute(xa), "xb": _to_compute(xb), "bs": bimg}
        im.update(cimages)
        in_maps.append(im)
    wcols = {name: arr.shape[1] for name, arr in images.items()}
    return in_maps, wcols


def kernel(x, padding_mask, params):
    global _PROG, LAST_RESULTS
    from concourse.bass_utils import run_bass_kernel_spmd

    in_maps, wcols = _prepare(x, padding_mask, params)
    if _PROG is None:
        _PROG = _build_program(wcols)

    trace = os.environ.get("BASS_KERNEL_TRACE", "0") == "1"
    res = run_bass_kernel_spmd(
        _PROG, in_maps, core_ids=list(range(NCORES)), trace=trace
    )
    LAST_RESULTS = res
    out = np.empty((B, NCLS), np.float32)
    for c in range(NCORES):
        out[c * BC : (c + 1) * BC] = res.results[c]["y"].T
    return out


# revision 14
# speedup vs baseline: 1.8451x; 1.2747x over previous
"""Trainium2 Bass kernel for nn_BaselineModel (sampling + MSResNet + FC head).

Contract: kernel(**inputs) takes FULL unsharded inputs (x [32,100,30,1000] f32,
padding_mask [32,100,30] bool, params pytree) and returns the FULL output
[32, 2] f32.  Internally: batch is sharded 4-per-core across 8 NeuronCores;
the electrode/trial sampling indices (which depend only on padding_mask and a
fixed PRNG key) are computed on host, the selected rows are gathered and laid
out as matmul-ready images, and the whole MSResNet + head runs on-device in
fp32r matmuls.
"""

import os
import numpy as np

B, E, TR, T = 32, 100, 30, 1000
N_ELECS, EMBED, NCLS = 25, 128, 2
NCORES, BC = 8, 4
L1, L2, L3 = 500, 250, 125
KS = (3, 5, 7)

LAST_RESULTS = None
_PROG = None
COMPUTE = os.environ.get("BASS_KERNEL_DT", "bf16")  # "bf16" | "f32r"


def _to_compute(a):
    if COMPUTE == "bf16":
        import ml_dtypes

        return np.ascontiguousarray(a).astype(ml_dtypes.bfloat16)
    return np.ascontiguousarray(a, np.float32)


# ---------------------------------------------------------------- host side


def _sample_indices(padding_mask):
    """Bit-exact replication of the reference's electrode/trial sampling."""
    import jax

    # IMPORTANT: no device/impl overrides here — must match the ambient code
    # path reference.py uses, which yields different streams than e.g.
    # running under jax.default_device(cpu).
    k1, k2 = jax.random.split(jax.random.key(42))
    eg = np.asarray(jax.random.uniform(k1, (B, E)), np.float32)
    tg = np.asarray(jax.random.uniform(k2, (B, N_ELECS, TR)), np.float32)
    vt_full = ~padding_mask
    valid_elec = vt_full.any(-1)
    scores = np.where(valid_elec, eg, np.float32(-1.0))
    sel = np.argsort(-scores, axis=-1, kind="stable")[:, :N_ELECS]
    vt = np.take_along_axis(vt_full, sel[:, :, None], axis=1)
    trial = np.argmax(np.where(vt, tg, np.float32(-1.0)), axis=-1)
    return sel, trial


class _Pack:
    """Packs [rows<=128, cols] f32 regions into a [128, W] image; regions with
    rows<=64 are paired top/bottom to halve DMA bytes."""

    def __init__(self):
        self.cols = 0
        self.regions = {}
        self.pending = []
        self.arrays = []

    def add(self, name, arr):
        arr = np.ascontiguousarray(arr, np.float32)
        r, c = arr.shape
        assert r <= 128
        # no top/bottom pairing: matmul requires lhsT.base_partition ==
        # rhs.base_partition, and all conv inputs live at base 0
        c0 = self.cols
        self.cols += c
        self._place(name, 0, c0, arr)

    def _place(self, name, r0, c0, arr):
        self.regions[name] = (r0, c0, arr.shape[0], arr.shape[1])
        self.arrays.append((r0, c0, arr))

    def image(self):
        img = np.zeros((128, max(self.cols, 1)), np.float32)
        for r0, c0, a in self.arrays:
            img[r0 : r0 + a.shape[0], c0 : c0 + a.shape[1]] = a
        return img


def _conv_lhst_tap(w, j):
    """[cout, cin, k] tap j -> lhsT [cin, cout]."""
    return np.ascontiguousarray(w[:, :, j].T)


def _groups(k):
    """Tap offsets o in [3-p, 3+p] grouped into even-start pairs + solos."""
    p = (k - 1) // 2
    o, out = 3 - p, []
    while o <= 3 + p:
        if o % 2 == 0 and o + 1 <= 3 + p:
            out.append((o, True))
            o += 2
        else:
            out.append((o, False))
            o += 1
    return out


def _pack_weights(params):
    p = {k: np.asarray(v, np.float32) for k, v in _flatten(params).items()}

    packs = {"w0": _Pack()}
    w0 = packs["w0"]
    stem = np.transpose(p["stem_w"], (2, 1, 0))  # [7, 25, 64]
    w0.add("stemA", stem[:5].reshape(125, 64))
    w0.add("stemB", stem[5:7].reshape(50, 64))
    w0.add("id64", np.eye(64, dtype=np.float32))
    fc = np.concatenate(
        [p["fc_w"][i * 128 : (i + 1) * 128] / 125.0 for i in range(3)], axis=1
    )
    w0.add("fc", fc)
    w0.add("cl1", p["cl1_w"])
    w0.add("cl2", p["cl2_w"])

    def paired(w, k):
        """Return (pairs [128, npair*cout], solos [64, nsolo*cout])."""
        cout = w.shape[0]
        pcv = (k - 1) // 2
        prs, sls = [], []
        for o, ispair in _groups(k):
            j = o - (3 - pcv)
            if ispair:
                blk = np.zeros((128, cout), np.float32)
                blk[0:64] = _conv_lhst_tap(w, j)
                blk[64:128] = _conv_lhst_tap(w, j + 1)
                prs.append(blk)
            else:
                sls.append(_conv_lhst_tap(w, j))
        pa = np.concatenate(prs, axis=1) if prs else np.zeros((128, 0), np.float32)
        sa = np.concatenate(sls, axis=1) if sls else np.zeros((64, 0), np.float32)
        return pa, sa

    for i, k in enumerate(KS):
        pk = _Pack()
        packs[f"w{k}"] = pk
        for cname, wkey in (("b1c1", f"br{i}.b1.w1"), ("b1c2", f"br{i}.b1.w2"),
                            ("b2c1", f"br{i}.b2.w1")):
            pa, sa = paired(p[wkey], k)
            if pa.shape[1]:
                pk.add(f"{cname}_p", pa)
            if sa.shape[1]:
                pk.add(f"{cname}_s", sa)
        # b2c2: full 128 contraction, one tap per group
        w2 = p[f"br{i}.b2.w2"]
        pk.add("b2c2", np.concatenate(
            [_conv_lhst_tap(w2, j) for j in range(k)], axis=1))
        pk.add("down", _conv_lhst_tap(p[f"br{i}.b2.wd"], 0))

    bimg = np.zeros((128, 16), np.float32)
    bimg[:64, 0] = p["stem_b"]
    for i in range(3):
        bimg[:64, 1 + 4 * i] = p[f"br{i}.b1.b1"]
        bimg[:64, 2 + 4 * i] = p[f"br{i}.b1.b2"]
        bimg[:, 3 + 4 * i] = p[f"br{i}.b2.b1"]
        bimg[:, 4 + 4 * i] = p[f"br{i}.b2.b2"] + p[f"br{i}.b2.bd"]
    bimg[:, 13] = p["fc_b"]
    bimg[:, 14] = p["cl1_b"]
    bimg[:NCLS, 15] = p["cl2_b"]

    images = {name: pk.image() for name, pk in packs.items()}
    regions = {name: pk.regions for name, pk in packs.items()}
    return images, regions, bimg


def _flatten(params):
    out = {}
    for k, v in params.items():
        if k == "branches":
            for i, br in enumerate(v):
                for bn, bd in br.items():
                    for wn, wv in bd.items():
                        out[f"br{i}.{bn}.{wn}"] = wv
        else:
            out[k] = v
    return out


def _stem_images(xg):
    """xg: [BC, 25, 1000] gathered rows for one core.  Returns xa [125, 2000]
    (taps 0-4) and xb [50, 2000] (taps 5-6), with column order
    (chunk, b, t_local), chunk = 125 output positions."""
    xp = np.zeros((BC, N_ELECS, T + 6), np.float32)
    xp[:, :, 3 : 3 + T] = xg
    xa = np.empty((125, BC * L1), np.float32)
    xb = np.empty((50, BC * L1), np.float32)
    for j in range(7):
        arr = xp[:, :, j : j + 2 * L1 : 2]  # [BC, 25, 500]
        # -> [25, chunk, b, tl] -> [25, 2000]
        a4 = arr.transpose(1, 0, 2).reshape(N_ELECS, BC, 4, 125)
        # column order (chunk, t_local, b): batch-interleaved inner dim
        a4 = np.ascontiguousarray(a4.transpose(0, 2, 3, 1)).reshape(N_ELECS, BC * L1)
        if j < 5:
            xa[j * 25 : (j + 1) * 25] = a4
        else:
            xb[(j - 5) * 25 : (j - 4) * 25] = a4
    return xa, xb


# -------------------------------------------------------------- device side


def _build_program(wcols):
    import concourse.mybir as mybir
    from concourse import bacc
    from concourse.tile import TileContext

    F32 = mybir.dt.float32
    F32R = mybir.dt.bfloat16 if COMPUTE == "bf16" else mybir.dt.float32r
    AF = mybir.ActivationFunctionType
    ALU = mybir.AluOpType

    nc = bacc.Bacc("TRN2", target_bir_lowering=False, num_devices=NCORES)

    d_xa = nc.dram_tensor("xa", [125, BC * L1], F32R, kind="ExternalInput")
    d_xb = nc.dram_tensor("xb", [50, BC * L1], F32R, kind="ExternalInput")
    d_bs = nc.dram_tensor("bs", [128, 16], F32, kind="ExternalInput")
    d_w = {
        name: nc.dram_tensor(name, [128, wcols[name]], F32R, kind="ExternalInput")
        for name in ("w0", "w3", "w5", "w7")
    }
    d_y = nc.dram_tensor("y", [NCLS, BC], F32, kind="ExternalOutput")

    with TileContext(nc) as tc:
        with (
            tc.tile_pool(name="const", bufs=1) as cpool,
            tc.tile_pool(name="act", bufs=1) as apool,
            tc.tile_pool(name="work", bufs=2) as wpool,
            tc.tile_pool(name="pp", bufs=6, space="PSUM") as ppool,
            tc.tile_pool(name="ph", bufs=2, space="PSUM") as phpool,
        ):
            dma_engines = [nc.sync, nc.scalar]
            _ei = [0]

            def dma(out, in_):
                dma_engines[_ei[0] % len(dma_engines)].dma_start(out=out, in_=in_)
                _ei[0] += 1

            # preload the ACT function table while DMAs run
            dummy = wpool.tile([1, 1], F32, tag="dummy")
            nc.vector.memset(dummy[:], 0.0)
            nc.scalar.activation(dummy[:], dummy[:], AF.Relu)

            xa_t = cpool.tile([125, BC * L1], F32R, tag="xa")
            xb_t = cpool.tile([50, BC * L1], F32R, tag="xb")
            bs_t = cpool.tile([128, 16], F32, tag="bs")
            w_t = {}
            for img in ("w0", "w3", "w5", "w7"):
                w_t[img] = cpool.tile([128, wcols[img]], F32R, tag=img,
                                      name=f"{img}_t")
            stem_end = _REGIONS["w0"]["id64"][1]
            dma(xa_t[:, 0:500], d_xa[:, 0:500])
            dma(w_t["w0"][:, 0:stem_end], d_w["w0"][:, 0:stem_end])
            dma(xb_t[:, 0:1000], d_xb[:, 0:1000])
            dma(bs_t[:], d_bs[:])
            for c in range(1, 4):
                dma(xa_t[:, 500 * c : 500 * (c + 1)],
                    d_xa[:, 500 * c : 500 * (c + 1)])
            dma(xb_t[:, 1000:2000], d_xb[:, 1000:2000])
            dma(w_t["w3"][:], d_w["w3"][:])
            dma(w_t["w5"][:], d_w["w5"][:])
            W7 = wcols["w7"]
            dma(w_t["w7"][:, 0 : W7 // 2], d_w["w7"][:, 0 : W7 // 2])
            dma(w_t["w7"][:, W7 // 2 :], d_w["w7"][:, W7 // 2 :])
            dma(w_t["w0"][:, stem_end:], d_w["w0"][:, stem_end:])

            def wreg(img, rname, rows, j, cout):
                r0, c0, rr, cc = _REGIONS[img][rname]
                return w_t[img][0:rows, c0 + j * cout : c0 + (j + 1) * cout]

            def bias(col, rows=128):
                return bs_t[0:rows, col : col + 1]

            # ---- persistent tiles, batch-interleaved (col = BC*t + b).
            # d-tiles: rows 0:64 = tensor, rows 64:128 = tensor shifted by one
            # t (filled by an SBUF->SBUF DMA), enabling 128-deep tap pairs.
            h1 = apool.tile([64, BC * (L1 + 2)], F32R, tag="h1")
            h2d = apool.tile([128, BC * (L2 + 6)], F32R, tag="h2d")
            y1d = {k: apool.tile([128, BC * (L2 + 6)], F32R, tag=f"y1d_{k}", name=f"y1d_{k}") for k in KS}
            y2d = {k: apool.tile([128, BC * (L2 + 6)], F32R, tag=f"y2d_{k}", name=f"y2d_{k}") for k in KS}
            z1 = {k: apool.tile([128, BC * (L3 + 6)], F32R, tag=f"z1_{k}", name=f"z1_{k}") for k in KS}
            feats32 = apool.tile([128, 12], F32, tag="feats32")
            feats = apool.tile([128, 12], F32R, tag="feats")

            def pad_memset(tile, parts, lp, interior_l, lpad):
                u32 = mybir.dt.uint32
                if lpad:
                    nc.vector.memset(tile[0:parts, 0 : BC * lpad].bitcast(u32), 0)
                rs = lpad + interior_l
                if rs < lp:
                    nc.vector.memset(tile[0:parts, BC * rs : BC * lp].bitcast(u32), 0)

            pad_memset(h1, 64, L1 + 2, L1, 1)
            pad_memset(h2d, 64, L2 + 6, L2, 3)
            for k in KS:
                pad_memset(y1d[k], 64, L2 + 6, L2, 3)
                pad_memset(y2d[k], 64, L2 + 6, L2, 3)
                pad_memset(z1[k], 128, L3 + 6, L3, 3)

            def shift_fill(dt_, lp):
                # bottom[t] = top[t+1]; last bottom column stays unread
                dma(dt_[64:128, 0 : BC * (lp - 1)], dt_[0:64, BC * 1 : BC * lp])

            # ---- stem
            for c in range(4):
                pt = ppool.tile([128, 500], F32, tag="pt")
                cs = slice(c * 500, (c + 1) * 500)
                nc.tensor.matmul(pt[0:64, :], lhsT=wreg("w0", "stemA", 125, 0, 64),
                                 rhs=xa_t[:, cs], start=True, stop=False)
                nc.tensor.matmul(pt[0:64, :], lhsT=wreg("w0", "stemB", 50, 0, 64),
                                 rhs=xb_t[:, cs], start=False, stop=True)
                nc.scalar.activation(
                    h1[0:64, BC * (1 + 125 * c) : BC * (126 + 125 * c)],
                    pt[0:64, :], AF.Relu, bias=bias(0, 64),
                )

            # ---- maxpool 3 s2 p1 into h2d top
            h1w = h1.rearrange("c (t b) -> c t b", b=BC)
            pm = wpool.tile([64, BC * 250], F32R, tag="pm")
            for half in range(2):
                t0 = 125 * half
                a = h1w[0:64, 2 * t0 + 0 : 2 * t0 + 250 : 2, :]
                bb = h1w[0:64, 2 * t0 + 1 : 2 * t0 + 251 : 2, :]
                cc = h1w[0:64, 2 * t0 + 2 : 2 * t0 + 252 : 2, :]
                pmv = pm[:, BC * t0 : BC * (t0 + 125)].rearrange(
                    "c (t b) -> c t b", b=BC)
                h2v = h2d[0:64, BC * (3 + t0) : BC * (3 + t0 + 125)].rearrange(
                    "c (t b) -> c t b", b=BC)
                nc.vector.tensor_tensor(pmv, a, bb, ALU.max)
                nc.vector.tensor_tensor(h2v, pmv, cc, ALU.max)
            shift_fill(h2d, L2 + 6)

            # ---- paired-tap conv over a d-tile (stride 1 contiguous slices)
            def conv_pair(img, cname, dt_, cout, k, Lout, out_t=None, out_off=0,
                          bias_col=None, epilogue=None, extra=None, stride=1):
                nch = (BC * Lout + 499) // 500
                cn = Lout // nch
                dt3 = dt_.rearrange("c (t b) -> c t b", b=BC)
                for c in range(nch):
                    pt = ppool.tile([128, 500], F32, tag="pt")
                    po = pt[0:cout, 0 : BC * cn]
                    groups = _groups(k)
                    # solos first: they read only the unshifted top half, so
                    # they can start before the shift DMA lands
                    order = [g for g in groups if not g[1]] + [g for g in groups if g[1]]
                    pidx = {}
                    np_, ns_ = 0, 0
                    for o, ispair in groups:
                        if ispair:
                            pidx[o] = np_; np_ += 1
                        else:
                            pidx[o] = ns_; ns_ += 1
                    for gi, (o, ispair) in enumerate(order):
                        lastg = gi == len(order) - 1
                        if ispair:
                            lhsT = wreg(img, f"{cname}_p", 128, pidx[o], cout)
                            rows = 128
                        else:
                            lhsT = wreg(img, f"{cname}_s", 64, pidx[o], cout)
                            rows = 64
                        if stride == 1:
                            rhs = dt_[0:rows, BC * (o + c * cn) : BC * (o + c * cn + cn)]
                        else:
                            rhs = dt3[0:rows, o : o + 2 * cn : 2, :]
                        nc.tensor.matmul(po, lhsT=lhsT, rhs=rhs,
                                         start=(gi == 0),
                                         stop=(lastg and extra is None))
                    if extra is not None:
                        extra(po, c, cn)
                    if epilogue == "act":
                        nc.scalar.activation(
                            out_t[0:cout, BC * (out_off + c * cn) : BC * (out_off + (c + 1) * cn)],
                            po, AF.Relu, bias=bias(bias_col, cout),
                        )
                    else:
                        epilogue(pt, c, cn)

            # ---- wave-interleaved branch stages
            for i, k in enumerate(KS):
                conv_pair(f"w{k}", "b1c1", h2d, 64, k, L2,
                          out_t=y1d[k], out_off=3, bias_col=1 + 4 * i,
                          epilogue="act")
            for k in KS:
                shift_fill(y1d[k], L2 + 6)

            for i, k in enumerate(KS):
                def sc_extra(po, c, cn):
                    nc.tensor.matmul(
                        po, lhsT=wreg("w0", "id64", 64, 0, 64),
                        rhs=h2d[0:64, BC * (3 + c * cn) : BC * (3 + (c + 1) * cn)],
                        start=False, stop=True,
                    )
                conv_pair(f"w{k}", "b1c2", y1d[k], 64, k, L2,
                          out_t=y2d[k], out_off=3, bias_col=2 + 4 * i,
                          epilogue="act", extra=sc_extra)
            for k in KS:
                shift_fill(y2d[k], L2 + 6)

            def b2c1_stage(i, k):
                conv_pair(f"w{k}", "b2c1", y2d[k], 128, k, L3,
                          out_t=z1[k], out_off=3, bias_col=3 + 4 * i,
                          epilogue="act", stride=2)

            def b2c2_stage(i, k):
                wk = f"w{k}"
                y2d3 = y2d[k].rearrange("c (t b) -> c t b", b=BC)

                def ds_extra(po, c, cn, _k=k, _y=y2d3):
                    nc.tensor.matmul(
                        po, lhsT=wreg(f"w{_k}", "down", 64, 0, 128),
                        rhs=_y[0:64, 3 : 253 : 2, :],
                        start=False, stop=True,
                    )

                def pool_epilogue(pt, c, cn, _i=i):
                    rt = wpool.tile([128, BC * L3], F32R, tag="rt")
                    nc.vector.tensor_scalar(
                        rt[:], pt[0:128, 0 : BC * L3], bias(4 + 4 * _i), 0.0,
                        ALU.add, ALU.max,
                    )
                    nc.vector.tensor_reduce(
                        feats32[:, _i * 4 : (_i + 1) * 4],
                        rt.rearrange("c (t b) -> c b t", b=BC),
                        mybir.AxisListType.X, ALU.add,
                    )
                    nc.vector.tensor_copy(
                        out=feats[:, _i * 4 : (_i + 1) * 4],
                        in_=feats32[:, _i * 4 : (_i + 1) * 4],
                    )

                # b2c2: full-128 contraction, one tap per matmul
                p = (k - 1) // 2
                pt = ppool.tile([128, 500], F32, tag="pt")
                po = pt[0:128, 0 : BC * L3]
                for j in range(k):
                    o = (3 - p) + j
                    rhs = z1[k][0:128, BC * o : BC * (o + L3)]
                    nc.tensor.matmul(po, lhsT=wreg(wk, "b2c2", 128, j, 128),
                                     rhs=rhs, start=(j == 0), stop=False)
                ds_extra(po, 0, L3)
                pool_epilogue(pt, 0, L3)

            # interleave so each b2c2 waits behind other ready work
            b2c1_stage(0, 3)
            b2c1_stage(1, 5)
            b2c2_stage(0, 3)
            b2c1_stage(2, 7)
            b2c2_stage(1, 5)
            b2c2_stage(2, 7)

            # ---- head
            pe = phpool.tile([128, 8], F32, tag="ph")
            for br in range(3):
                nc.tensor.matmul(
                    pe[:, 0:4], lhsT=wreg("w0", "fc", 128, br, 128),
                    rhs=feats[:, br * 4 : (br + 1) * 4],
                    start=(br == 0), stop=(br == 2),
                )
            embt = wpool.tile([128, 4], F32R, tag="embt")
            nc.scalar.activation(embt[:], pe[:, 0:4], AF.Identity, bias=bias(13))

            ph2 = phpool.tile([128, 8], F32, tag="ph")
            nc.tensor.matmul(ph2[:, 0:4], lhsT=wreg("w0", "cl1", 128, 0, 128),
                             rhs=embt[:], start=True, stop=True)
            hct = wpool.tile([128, 4], F32R, tag="hct")
            nc.scalar.activation(hct[:], ph2[:, 0:4], AF.Relu, bias=bias(14))

            ph3 = phpool.tile([128, 8], F32, tag="ph")
            nc.tensor.matmul(ph3[0:NCLS, 0:4], lhsT=wreg("w0", "cl2", 128, 0, NCLS),
                             rhs=hct[:], start=True, stop=True)
            outt = wpool.tile([NCLS, 4], F32, tag="outt")
            nc.scalar.activation(outt[:], ph3[0:NCLS, 0:4], AF.Identity,
                                 bias=bias(15, NCLS))
            nc.sync.dma_start(out=d_y[:], in_=outt[:])

    nc.compile()
    return nc


_REGIONS = None


def _prepare(x, padding_mask, params):
    global _REGIONS
    x = np.asarray(x, np.float32)
    padding_mask = np.asarray(padding_mask, bool)
    sel, trial = _sample_indices(padding_mask)
    images, regions, bimg = _pack_weights(params)
    cimages = {k: _to_compute(v) for k, v in images.items()}
    _REGIONS = regions

    in_maps = []
    for c in range(NCORES):
        bs = slice(c * BC, (c + 1) * BC)
        xs = x[bs]
        xg = xs[np.arange(BC)[:, None], sel[bs], trial[bs]]  # [BC, 25, 1000]
        xa, xb = _stem_images(xg)
        im = {"xa": _to_compute(xa), "xb": _to_compute(xb), "bs": bimg}
        im.update(cimages)
        in_maps.append(im)
    wcols = {name: arr.shape[1] for name, arr in images.items()}
    return in_maps, wcols


def kernel(x, padding_mask, params):
    global _PROG, LAST_RESULTS
    from concourse.bass_utils import run_bass_kernel_spmd

    in_maps, wcols = _prepare(x, padding_mask, params)
    if _PROG is None:
        _PROG = _build_program(wcols)

    trace = os.environ.get("BASS_KERNEL_TRACE", "0") == "1"
    res = run_bass_kernel_spmd(
        _PROG, in_maps, core_ids=list(range(NCORES)), trace=trace
    )
    LAST_RESULTS = res
    out = np.empty((B, NCLS), np.float32)
    for c in range(NCORES):
        out[c * BC : (c + 1) * BC] = res.results[c]["y"].T
    return out


# revision 15
# speedup vs baseline: 1.8712x; 1.0141x over previous
"""Trainium2 Bass kernel for nn_BaselineModel (sampling + MSResNet + FC head).

Contract: kernel(**inputs) takes FULL unsharded inputs (x [32,100,30,1000] f32,
padding_mask [32,100,30] bool, params pytree) and returns the FULL output
[32, 2] f32.  Internally: batch is sharded 4-per-core across 8 NeuronCores;
the electrode/trial sampling indices (which depend only on padding_mask and a
fixed PRNG key) are computed on host, the selected rows are gathered and laid
out as matmul-ready images, and the whole MSResNet + head runs on-device in
fp32r matmuls.
"""

import os
import numpy as np

B, E, TR, T = 32, 100, 30, 1000
N_ELECS, EMBED, NCLS = 25, 128, 2
NCORES, BC = 8, 4
L1, L2, L3 = 500, 250, 125
KS = (3, 5, 7)

LAST_RESULTS = None
_PROG = None
COMPUTE = os.environ.get("BASS_KERNEL_DT", "bf16")  # "bf16" | "f32r"


def _to_compute(a):
    if COMPUTE == "bf16":
        import ml_dtypes

        return np.ascontiguousarray(a).astype(ml_dtypes.bfloat16)
    return np.ascontiguousarray(a, np.float32)


# ---------------------------------------------------------------- host side


def _sample_indices(padding_mask):
    """Bit-exact replication of the reference's electrode/trial sampling."""
    import jax

    # IMPORTANT: no device/impl overrides here — must match the ambient code
    # path reference.py uses, which yields different streams than e.g.
    # running under jax.default_device(cpu).
    k1, k2 = jax.random.split(jax.random.key(42))
    eg = np.asarray(jax.random.uniform(k1, (B, E)), np.float32)
    tg = np.asarray(jax.random.uniform(k2, (B, N_ELECS, TR)), np.float32)
    vt_full = ~padding_mask
    valid_elec = vt_full.any(-1)
    scores = np.where(valid_elec, eg, np.float32(-1.0))
    sel = np.argsort(-scores, axis=-1, kind="stable")[:, :N_ELECS]
    vt = np.take_along_axis(vt_full, sel[:, :, None], axis=1)
    trial = np.argmax(np.where(vt, tg, np.float32(-1.0)), axis=-1)
    return sel, trial


class _Pack:
    """Packs [rows<=128, cols] f32 regions into a [128, W] image; regions with
    rows<=64 are paired top/bottom to halve DMA bytes."""

    def __init__(self):
        self.cols = 0
        self.regions = {}
        self.pending = []
        self.arrays = []

    def add(self, name, arr):
        arr = np.ascontiguousarray(arr, np.float32)
        r, c = arr.shape
        assert r <= 128
        # no top/bottom pairing: matmul requires lhsT.base_partition ==
        # rhs.base_partition, and all conv inputs live at base 0
        c0 = self.cols
        self.cols += c
        self._place(name, 0, c0, arr)

    def _place(self, name, r0, c0, arr):
        self.regions[name] = (r0, c0, arr.shape[0], arr.shape[1])
        self.arrays.append((r0, c0, arr))

    def image(self):
        img = np.zeros((128, max(self.cols, 1)), np.float32)
        for r0, c0, a in self.arrays:
            img[r0 : r0 + a.shape[0], c0 : c0 + a.shape[1]] = a
        return img


def _conv_lhst_tap(w, j):
    """[cout, cin, k] tap j -> lhsT [cin, cout]."""
    return np.ascontiguousarray(w[:, :, j].T)


def _groups(k):
    """Tap offsets o in [3-p, 3+p] grouped into even-start pairs + solos."""
    p = (k - 1) // 2
    o, out = 3 - p, []
    while o <= 3 + p:
        if o % 2 == 0 and o + 1 <= 3 + p:
            out.append((o, True))
            o += 2
        else:
            out.append((o, False))
            o += 1
    return out


def _pack_weights(params):
    p = {k: np.asarray(v, np.float32) for k, v in _flatten(params).items()}

    packs = {"w0": _Pack()}
    w0 = packs["w0"]
    stem = np.transpose(p["stem_w"], (2, 1, 0))  # [7, 25, 64]
    w0.add("stemA", stem[:5].reshape(125, 64))
    w0.add("stemB", stem[5:7].reshape(50, 64))
    w0.add("id64", np.eye(64, dtype=np.float32))
    fc = np.concatenate(
        [p["fc_w"][i * 128 : (i + 1) * 128] / 125.0 for i in range(3)], axis=1
    )
    w0.add("fc", fc)
    w0.add("cl1", p["cl1_w"])
    w0.add("cl2", p["cl2_w"])

    def paired(w, k):
        """Return (pairs [128, npair*cout], solos [64, nsolo*cout])."""
        cout = w.shape[0]
        pcv = (k - 1) // 2
        prs, sls = [], []
        for o, ispair in _groups(k):
            j = o - (3 - pcv)
            if ispair:
                blk = np.zeros((128, cout), np.float32)
                blk[0:64] = _conv_lhst_tap(w, j)
                blk[64:128] = _conv_lhst_tap(w, j + 1)
                prs.append(blk)
            else:
                sls.append(_conv_lhst_tap(w, j))
        pa = np.concatenate(prs, axis=1) if prs else np.zeros((128, 0), np.float32)
        sa = np.concatenate(sls, axis=1) if sls else np.zeros((64, 0), np.float32)
        return pa, sa

    for i, k in enumerate(KS):
        pk = _Pack()
        packs[f"w{k}"] = pk
        for cname, wkey in (("b1c1", f"br{i}.b1.w1"), ("b1c2", f"br{i}.b1.w2"),
                            ("b2c1", f"br{i}.b2.w1")):
            pa, sa = paired(p[wkey], k)
            if pa.shape[1]:
                pk.add(f"{cname}_p", pa)
            if sa.shape[1]:
                pk.add(f"{cname}_s", sa)
        # b2c2: full 128 contraction, one tap per group
        w2 = p[f"br{i}.b2.w2"]
        pk.add("b2c2", np.concatenate(
            [_conv_lhst_tap(w2, j) for j in range(k)], axis=1))
        pk.add("down", _conv_lhst_tap(p[f"br{i}.b2.wd"], 0))

    bimg = np.zeros((128, 16), np.float32)
    bimg[:64, 0] = p["stem_b"]
    for i in range(3):
        bimg[:64, 1 + 4 * i] = p[f"br{i}.b1.b1"]
        bimg[:64, 2 + 4 * i] = p[f"br{i}.b1.b2"]
        bimg[:, 3 + 4 * i] = p[f"br{i}.b2.b1"]
        bimg[:, 4 + 4 * i] = p[f"br{i}.b2.b2"] + p[f"br{i}.b2.bd"]
    bimg[:, 13] = p["fc_b"]
    bimg[:, 14] = p["cl1_b"]
    bimg[:NCLS, 15] = p["cl2_b"]

    images = {name: pk.image() for name, pk in packs.items()}
    regions = {name: pk.regions for name, pk in packs.items()}
    return images, regions, bimg


def _flatten(params):
    out = {}
    for k, v in params.items():
        if k == "branches":
            for i, br in enumerate(v):
                for bn, bd in br.items():
                    for wn, wv in bd.items():
                        out[f"br{i}.{bn}.{wn}"] = wv
        else:
            out[k] = v
    return out


def _stem_images(xg):
    """xg: [BC, 25, 1000] gathered rows for one core.  Returns xa [125, 2000]
    (taps 0-4) and xb [50, 2000] (taps 5-6), with column order
    (chunk, b, t_local), chunk = 125 output positions."""
    xp = np.zeros((BC, N_ELECS, T + 6), np.float32)
    xp[:, :, 3 : 3 + T] = xg
    xa = np.empty((125, BC * L1), np.float32)
    xb = np.empty((50, BC * L1), np.float32)
    for j in range(7):
        arr = xp[:, :, j : j + 2 * L1 : 2]  # [BC, 25, 500]
        # -> [25, chunk, b, tl] -> [25, 2000]
        a4 = arr.transpose(1, 0, 2).reshape(N_ELECS, BC, 4, 125)
        # column order (chunk, t_local, b): batch-interleaved inner dim
        a4 = np.ascontiguousarray(a4.transpose(0, 2, 3, 1)).reshape(N_ELECS, BC * L1)
        if j < 5:
            xa[j * 25 : (j + 1) * 25] = a4
        else:
            xb[(j - 5) * 25 : (j - 4) * 25] = a4
    return xa, xb


# -------------------------------------------------------------- device side


def _build_program(wcols):
    import concourse.mybir as mybir
    from concourse import bacc
    from concourse.tile import TileContext

    F32 = mybir.dt.float32
    F32R = mybir.dt.bfloat16 if COMPUTE == "bf16" else mybir.dt.float32r
    AF = mybir.ActivationFunctionType
    ALU = mybir.AluOpType

    nc = bacc.Bacc("TRN2", target_bir_lowering=False, num_devices=NCORES)

    d_xa = nc.dram_tensor("xa", [125, BC * L1], F32R, kind="ExternalInput")
    d_xb = nc.dram_tensor("xb", [50, BC * L1], F32R, kind="ExternalInput")
    d_bs = nc.dram_tensor("bs", [128, 16], F32, kind="ExternalInput")
    d_w = {
        name: nc.dram_tensor(name, [128, wcols[name]], F32R, kind="ExternalInput")
        for name in ("w0", "w3", "w5", "w7")
    }
    d_y = nc.dram_tensor("y", [NCLS, BC], F32, kind="ExternalOutput")

    with TileContext(nc) as tc:
        with (
            tc.tile_pool(name="const", bufs=1) as cpool,
            tc.tile_pool(name="act", bufs=1) as apool,
            tc.tile_pool(name="work", bufs=2) as wpool,
            tc.tile_pool(name="pp", bufs=6, space="PSUM") as ppool,
            tc.tile_pool(name="ph", bufs=2, space="PSUM") as phpool,
        ):
            dma_engines = [nc.sync, nc.scalar]
            _ei = [0]

            def dma(out, in_):
                dma_engines[_ei[0] % len(dma_engines)].dma_start(out=out, in_=in_)
                _ei[0] += 1

            # preload the ACT function table while DMAs run
            dummy = wpool.tile([1, 1], F32, tag="dummy")
            nc.vector.memset(dummy[:], 0.0)
            nc.scalar.activation(dummy[:], dummy[:], AF.Relu)

            xa_t = cpool.tile([125, BC * L1], F32R, tag="xa")
            xb_t = cpool.tile([50, BC * L1], F32R, tag="xb")
            bs_t = cpool.tile([128, 16], F32, tag="bs")
            w_t = {}
            for img in ("w0", "w3", "w5", "w7"):
                w_t[img] = cpool.tile([128, wcols[img]], F32R, tag=img,
                                      name=f"{img}_t")
            stem_end = _REGIONS["w0"]["id64"][1]
            # b1-weight column boundary inside each branch image
            b1_end = {k: _REGIONS[f"w{k}"]["b2c1_p"][1] for k in KS}
            dma(xa_t[:, 0:250], d_xa[:, 0:250])
            dma(w_t["w0"][:, 0:stem_end], d_w["w0"][:, 0:stem_end])
            dma(xa_t[:, 250:500], d_xa[:, 250:500])
            dma(xb_t[:, 0:500], d_xb[:, 0:500])
            dma(xa_t[:, 500:1000], d_xa[:, 500:1000])
            dma(bs_t[:], d_bs[:])
            dma(xa_t[:, 1000:1500], d_xa[:, 1000:1500])
            dma(xb_t[:, 500:1000], d_xb[:, 500:1000])
            dma(xa_t[:, 1500:2000], d_xa[:, 1500:2000])
            dma(xb_t[:, 1000:2000], d_xb[:, 1000:2000])
            for k in KS:
                dma(w_t[f"w{k}"][:, 0 : b1_end[k]], d_w[f"w{k}"][:, 0 : b1_end[k]])
            for k in KS:
                dma(w_t[f"w{k}"][:, b1_end[k] :], d_w[f"w{k}"][:, b1_end[k] :])
            dma(w_t["w0"][:, stem_end:], d_w["w0"][:, stem_end:])

            def wreg(img, rname, rows, j, cout):
                r0, c0, rr, cc = _REGIONS[img][rname]
                return w_t[img][0:rows, c0 + j * cout : c0 + (j + 1) * cout]

            def bias(col, rows=128):
                return bs_t[0:rows, col : col + 1]

            # ---- persistent tiles, batch-interleaved (col = BC*t + b).
            # d-tiles: rows 0:64 = tensor, rows 64:128 = tensor shifted by one
            # t (filled by an SBUF->SBUF DMA), enabling 128-deep tap pairs.
            h1 = apool.tile([64, BC * (L1 + 2)], F32R, tag="h1")
            h2d = apool.tile([128, BC * (L2 + 6)], F32R, tag="h2d")
            y1d = {k: apool.tile([128, BC * (L2 + 6)], F32R, tag=f"y1d_{k}", name=f"y1d_{k}") for k in KS}
            y2d = {k: apool.tile([128, BC * (L2 + 6)], F32R, tag=f"y2d_{k}", name=f"y2d_{k}") for k in KS}
            z1 = {k: apool.tile([128, BC * (L3 + 6)], F32R, tag=f"z1_{k}", name=f"z1_{k}") for k in KS}
            feats32 = apool.tile([128, 12], F32, tag="feats32")
            feats = apool.tile([128, 12], F32R, tag="feats")

            def pad_memset(tile, parts, lp, interior_l, lpad):
                u32 = mybir.dt.uint32
                if lpad:
                    nc.vector.memset(tile[0:parts, 0 : BC * lpad].bitcast(u32), 0)
                rs = lpad + interior_l
                if rs < lp:
                    nc.vector.memset(tile[0:parts, BC * rs : BC * lp].bitcast(u32), 0)

            pad_memset(h1, 64, L1 + 2, L1, 1)
            pad_memset(h2d, 64, L2 + 6, L2, 3)
            for k in KS:
                pad_memset(y1d[k], 64, L2 + 6, L2, 3)
                pad_memset(y2d[k], 64, L2 + 6, L2, 3)
                pad_memset(z1[k], 128, L3 + 6, L3, 3)

            def shift_fill(dt_, lp):
                # bottom[t] = top[t+1]; last bottom column stays unread
                dma(dt_[64:128, 0 : BC * (lp - 1)], dt_[0:64, BC * 1 : BC * lp])

            # ---- stem
            for c in range(4):
                pt = ppool.tile([128, 500], F32, tag="pt")
                cs = slice(c * 500, (c + 1) * 500)
                nc.tensor.matmul(pt[0:64, :], lhsT=wreg("w0", "stemA", 125, 0, 64),
                                 rhs=xa_t[:, cs], start=True, stop=False)
                nc.tensor.matmul(pt[0:64, :], lhsT=wreg("w0", "stemB", 50, 0, 64),
                                 rhs=xb_t[:, cs], start=False, stop=True)
                nc.scalar.activation(
                    h1[0:64, BC * (1 + 125 * c) : BC * (126 + 125 * c)],
                    pt[0:64, :], AF.Relu, bias=bias(0, 64),
                )

            # ---- maxpool 3 s2 p1 into h2d top
            h1w = h1.rearrange("c (t b) -> c t b", b=BC)
            pm = wpool.tile([64, BC * 250], F32R, tag="pm")
            for half in range(2):
                t0 = 125 * half
                a = h1w[0:64, 2 * t0 + 0 : 2 * t0 + 250 : 2, :]
                bb = h1w[0:64, 2 * t0 + 1 : 2 * t0 + 251 : 2, :]
                cc = h1w[0:64, 2 * t0 + 2 : 2 * t0 + 252 : 2, :]
                pmv = pm[:, BC * t0 : BC * (t0 + 125)].rearrange(
                    "c (t b) -> c t b", b=BC)
                h2v = h2d[0:64, BC * (3 + t0) : BC * (3 + t0 + 125)].rearrange(
                    "c (t b) -> c t b", b=BC)
                nc.vector.tensor_tensor(pmv, a, bb, ALU.max)
                nc.vector.tensor_tensor(h2v, pmv, cc, ALU.max)
            shift_fill(h2d, L2 + 6)

            # ---- paired-tap conv over a d-tile (stride 1 contiguous slices)
            def conv_pair(img, cname, dt_, cout, k, Lout, out_t=None, out_off=0,
                          bias_col=None, epilogue=None, extra=None, stride=1):
                nch = (BC * Lout + 499) // 500
                cn = Lout // nch
                dt3 = dt_.rearrange("c (t b) -> c t b", b=BC)
                for c in range(nch):
                    pt = ppool.tile([128, 500], F32, tag="pt")
                    po = pt[0:cout, 0 : BC * cn]
                    groups = _groups(k)
                    # solos first: they read only the unshifted top half, so
                    # they can start before the shift DMA lands
                    order = [g for g in groups if not g[1]] + [g for g in groups if g[1]]
                    pidx = {}
                    np_, ns_ = 0, 0
                    for o, ispair in groups:
                        if ispair:
                            pidx[o] = np_; np_ += 1
                        else:
                            pidx[o] = ns_; ns_ += 1
                    for gi, (o, ispair) in enumerate(order):
                        lastg = gi == len(order) - 1
                        if ispair:
                            lhsT = wreg(img, f"{cname}_p", 128, pidx[o], cout)
                            rows = 128
                        else:
                            lhsT = wreg(img, f"{cname}_s", 64, pidx[o], cout)
                            rows = 64
                        if stride == 1:
                            rhs = dt_[0:rows, BC * (o + c * cn) : BC * (o + c * cn + cn)]
                        else:
                            rhs = dt3[0:rows, o : o + 2 * cn : 2, :]
                        nc.tensor.matmul(po, lhsT=lhsT, rhs=rhs,
                                         start=(gi == 0),
                                         stop=(lastg and extra is None))
                    if extra is not None:
                        extra(po, c, cn)
                    if epilogue == "act":
                        nc.scalar.activation(
                            out_t[0:cout, BC * (out_off + c * cn) : BC * (out_off + (c + 1) * cn)],
                            po, AF.Relu, bias=bias(bias_col, cout),
                        )
                    else:
                        epilogue(pt, c, cn)

            # ---- wave-interleaved branch stages
            for i, k in enumerate(KS):
                conv_pair(f"w{k}", "b1c1", h2d, 64, k, L2,
                          out_t=y1d[k], out_off=3, bias_col=1 + 4 * i,
                          epilogue="act")
            for k in KS:
                shift_fill(y1d[k], L2 + 6)

            for i, k in enumerate(KS):
                def sc_extra(po, c, cn):
                    nc.tensor.matmul(
                        po, lhsT=wreg("w0", "id64", 64, 0, 64),
                        rhs=h2d[0:64, BC * (3 + c * cn) : BC * (3 + (c + 1) * cn)],
                        start=False, stop=True,
                    )
                conv_pair(f"w{k}", "b1c2", y1d[k], 64, k, L2,
                          out_t=y2d[k], out_off=3, bias_col=2 + 4 * i,
                          epilogue="act", extra=sc_extra)
            for k in KS:
                shift_fill(y2d[k], L2 + 6)

            def b2c1_stage(i, k):
                conv_pair(f"w{k}", "b2c1", y2d[k], 128, k, L3,
                          out_t=z1[k], out_off=3, bias_col=3 + 4 * i,
                          epilogue="act", stride=2)

            def b2c2_stage(i, k):
                wk = f"w{k}"
                y2d3 = y2d[k].rearrange("c (t b) -> c t b", b=BC)

                def ds_extra(po, c, cn, _k=k, _y=y2d3):
                    nc.tensor.matmul(
                        po, lhsT=wreg(f"w{_k}", "down", 64, 0, 128),
                        rhs=_y[0:64, 3 : 253 : 2, :],
                        start=False, stop=True,
                    )

                def pool_epilogue(pt, c, cn, _i=i):
                    rt = wpool.tile([128, BC * L3], F32R, tag="rt")
                    nc.vector.tensor_scalar(
                        rt[:], pt[0:128, 0 : BC * L3], bias(4 + 4 * _i), 0.0,
                        ALU.add, ALU.max,
                    )
                    nc.vector.tensor_reduce(
                        feats32[:, _i * 4 : (_i + 1) * 4],
                        rt.rearrange("c (t b) -> c b t", b=BC),
                        mybir.AxisListType.X, ALU.add,
                    )
                    nc.vector.tensor_copy(
                        out=feats[:, _i * 4 : (_i + 1) * 4],
                        in_=feats32[:, _i * 4 : (_i + 1) * 4],
                    )

                # b2c2: full-128 contraction, one tap per matmul
                p = (k - 1) // 2
                pt = ppool.tile([128, 500], F32, tag="pt")
                po = pt[0:128, 0 : BC * L3]
                for j in range(k):
                    o = (3 - p) + j
                    rhs = z1[k][0:128, BC * o : BC * (o + L3)]
                    nc.tensor.matmul(po, lhsT=wreg(wk, "b2c2", 128, j, 128),
                                     rhs=rhs, start=(j == 0), stop=False)
                ds_extra(po, 0, L3)
                pool_epilogue(pt, 0, L3)

            # interleave so each b2c2 waits behind other ready work
            b2c1_stage(0, 3)
            b2c1_stage(1, 5)
            b2c2_stage(0, 3)
            b2c1_stage(2, 7)
            b2c2_stage(1, 5)
            b2c2_stage(2, 7)

            # ---- head
            pe = phpool.tile([128, 8], F32, tag="ph")
            for br in range(3):
                nc.tensor.matmul(
                    pe[:, 0:4], lhsT=wreg("w0", "fc", 128, br, 128),
                    rhs=feats[:, br * 4 : (br + 1) * 4],
                    start=(br == 0), stop=(br == 2),
                )
            embt = wpool.tile([128, 4], F32R, tag="embt")
            nc.scalar.activation(embt[:], pe[:, 0:4], AF.Identity, bias=bias(13))

            ph2 = phpool.tile([128, 8], F32, tag="ph")
            nc.tensor.matmul(ph2[:, 0:4], lhsT=wreg("w0", "cl1", 128, 0, 128),
                             rhs=embt[:], start=True, stop=True)
            hct = wpool.tile([128, 4], F32R, tag="hct")
            nc.scalar.activation(hct[:], ph2[:, 0:4], AF.Relu, bias=bias(14))

            ph3 = phpool.tile([128, 8], F32, tag="ph")
            nc.tensor.matmul(ph3[0:NCLS, 0:4], lhsT=wreg("w0", "cl2", 128, 0, NCLS),
                             rhs=hct[:], start=True, stop=True)
            outt = wpool.tile([NCLS, 4], F32, tag="outt")
            nc.scalar.activation(outt[:], ph3[0:NCLS, 0:4], AF.Identity,
                                 bias=bias(15, NCLS))
            nc.sync.dma_start(out=d_y[:], in_=outt[:])

    nc.compile()
    return nc


_REGIONS = None


def _prepare(x, padding_mask, params):
    global _REGIONS
    x = np.asarray(x, np.float32)
    padding_mask = np.asarray(padding_mask, bool)
    sel, trial = _sample_indices(padding_mask)
    images, regions, bimg = _pack_weights(params)
    cimages = {k: _to_compute(v) for k, v in images.items()}
    _REGIONS = regions

    in_maps = []
    for c in range(NCORES):
        bs = slice(c * BC, (c + 1) * BC)
        xs = x[bs]
        xg = xs[np.arange(BC)[:, None], sel[bs], trial[bs]]  # [BC, 25, 1000]
        xa, xb = _stem_images(xg)
        im = {"xa": _to_compute(xa), "xb": _to_compute(xb), "bs": bimg}
        im.update(cimages)
        in_maps.append(im)
    wcols = {name: arr.shape[1] for name, arr in images.items()}
    return in_maps, wcols


def kernel(x, padding_mask, params):
    global _PROG, LAST_RESULTS
    from concourse.bass_utils import run_bass_kernel_spmd

    in_maps, wcols = _prepare(x, padding_mask, params)
    if _PROG is None:
        _PROG = _build_program(wcols)

    trace = os.environ.get("BASS_KERNEL_TRACE", "0") == "1"
    res = run_bass_kernel_spmd(
        _PROG, in_maps, core_ids=list(range(NCORES)), trace=trace
    )
    LAST_RESULTS = res
    out = np.empty((B, NCLS), np.float32)
    for c in range(NCORES):
        out[c * BC : (c + 1) * BC] = res.results[c]["y"].T
    return out


# revision 16
# speedup vs baseline: 1.9790x; 1.0576x over previous
"""Trainium2 Bass kernel for nn_BaselineModel (sampling + MSResNet + FC head).

Contract: kernel(**inputs) takes FULL unsharded inputs (x [32,100,30,1000] f32,
padding_mask [32,100,30] bool, params pytree) and returns the FULL output
[32, 2] f32.  Internally: batch is sharded 4-per-core across 8 NeuronCores;
the electrode/trial sampling indices (which depend only on padding_mask and a
fixed PRNG key) are computed on host, the selected rows are gathered and laid
out as matmul-ready images, and the whole MSResNet + head runs on-device in
fp32r matmuls.
"""

import os
import numpy as np

B, E, TR, T = 32, 100, 30, 1000
N_ELECS, EMBED, NCLS = 25, 128, 2
NCORES, BC = 8, 4
L1, L2, L3 = 500, 250, 125
KS = (3, 5, 7)

LAST_RESULTS = None
_PROG = None
COMPUTE = os.environ.get("BASS_KERNEL_DT", "bf16")  # "bf16" | "f32r"


def _to_compute(a):
    if COMPUTE == "bf16":
        import ml_dtypes

        return np.ascontiguousarray(a).astype(ml_dtypes.bfloat16)
    return np.ascontiguousarray(a, np.float32)


# ---------------------------------------------------------------- host side


def _sample_indices(padding_mask):
    """Bit-exact replication of the reference's electrode/trial sampling."""
    import jax

    # IMPORTANT: no device/impl overrides here — must match the ambient code
    # path reference.py uses, which yields different streams than e.g.
    # running under jax.default_device(cpu).
    k1, k2 = jax.random.split(jax.random.key(42))
    eg = np.asarray(jax.random.uniform(k1, (B, E)), np.float32)
    tg = np.asarray(jax.random.uniform(k2, (B, N_ELECS, TR)), np.float32)
    vt_full = ~padding_mask
    valid_elec = vt_full.any(-1)
    scores = np.where(valid_elec, eg, np.float32(-1.0))
    sel = np.argsort(-scores, axis=-1, kind="stable")[:, :N_ELECS]
    vt = np.take_along_axis(vt_full, sel[:, :, None], axis=1)
    trial = np.argmax(np.where(vt, tg, np.float32(-1.0)), axis=-1)
    return sel, trial


class _Pack:
    """Packs [rows<=128, cols] f32 regions into a [128, W] image; regions with
    rows<=64 are paired top/bottom to halve DMA bytes."""

    def __init__(self):
        self.cols = 0
        self.regions = {}
        self.pending = []
        self.arrays = []

    def add(self, name, arr):
        arr = np.ascontiguousarray(arr, np.float32)
        r, c = arr.shape
        assert r <= 128
        # no top/bottom pairing: matmul requires lhsT.base_partition ==
        # rhs.base_partition, and all conv inputs live at base 0
        c0 = self.cols
        self.cols += c
        self._place(name, 0, c0, arr)

    def _place(self, name, r0, c0, arr):
        self.regions[name] = (r0, c0, arr.shape[0], arr.shape[1])
        self.arrays.append((r0, c0, arr))

    def image(self):
        img = np.zeros((128, max(self.cols, 1)), np.float32)
        for r0, c0, a in self.arrays:
            img[r0 : r0 + a.shape[0], c0 : c0 + a.shape[1]] = a
        return img


def _conv_lhst_tap(w, j):
    """[cout, cin, k] tap j -> lhsT [cin, cout]."""
    return np.ascontiguousarray(w[:, :, j].T)


def _groups(k):
    """Tap offsets o in [3-p, 3+p] grouped into even-start pairs + solos."""
    p = (k - 1) // 2
    o, out = 3 - p, []
    while o <= 3 + p:
        if o % 2 == 0 and o + 1 <= 3 + p:
            out.append((o, True))
            o += 2
        else:
            out.append((o, False))
            o += 1
    return out


def _pack_weights(params):
    p = {k: np.asarray(v, np.float32) for k, v in _flatten(params).items()}

    packs = {"w0": _Pack()}
    w0 = packs["w0"]
    stem = np.transpose(p["stem_w"], (2, 1, 0))  # [7, 25, 64]
    w0.add("stemA", stem[:5].reshape(125, 64))
    w0.add("stemB", stem[5:7].reshape(50, 64))
    w0.add("id64", np.eye(64, dtype=np.float32))
    fc = np.concatenate(
        [p["fc_w"][i * 128 : (i + 1) * 128] / 125.0 for i in range(3)], axis=1
    )
    w0.add("fc", fc)
    w0.add("cl1", p["cl1_w"])
    w0.add("cl2", p["cl2_w"])

    def paired(w, k):
        """Return (pairs [128, npair*cout], solos [64, nsolo*cout])."""
        cout = w.shape[0]
        pcv = (k - 1) // 2
        prs, sls = [], []
        for o, ispair in _groups(k):
            j = o - (3 - pcv)
            if ispair:
                blk = np.zeros((128, cout), np.float32)
                blk[0:64] = _conv_lhst_tap(w, j)
                blk[64:128] = _conv_lhst_tap(w, j + 1)
                prs.append(blk)
            else:
                sls.append(_conv_lhst_tap(w, j))
        pa = np.concatenate(prs, axis=1) if prs else np.zeros((128, 0), np.float32)
        sa = np.concatenate(sls, axis=1) if sls else np.zeros((64, 0), np.float32)
        return pa, sa

    for i, k in enumerate(KS):
        pk = _Pack()
        packs[f"w{k}"] = pk
        for cname, wkey in (("b1c1", f"br{i}.b1.w1"), ("b1c2", f"br{i}.b1.w2"),
                            ("b2c1", f"br{i}.b2.w1")):
            pa, sa = paired(p[wkey], k)
            if pa.shape[1]:
                pk.add(f"{cname}_p", pa)
            if sa.shape[1]:
                pk.add(f"{cname}_s", sa)
        # b2c2: full 128 contraction, one tap per group
        w2 = p[f"br{i}.b2.w2"]
        pk.add("b2c2", np.concatenate(
            [_conv_lhst_tap(w2, j) for j in range(k)], axis=1))
        pk.add("down", _conv_lhst_tap(p[f"br{i}.b2.wd"], 0))

    bimg = np.zeros((128, 16), np.float32)
    bimg[:64, 0] = p["stem_b"]
    for i in range(3):
        bimg[:64, 1 + 4 * i] = p[f"br{i}.b1.b1"]
        bimg[:64, 2 + 4 * i] = p[f"br{i}.b1.b2"]
        bimg[:, 3 + 4 * i] = p[f"br{i}.b2.b1"]
        bimg[:, 4 + 4 * i] = p[f"br{i}.b2.b2"] + p[f"br{i}.b2.bd"]
    bimg[:, 13] = p["fc_b"]
    bimg[:, 14] = p["cl1_b"]
    bimg[:NCLS, 15] = p["cl2_b"]

    images = {name: pk.image() for name, pk in packs.items()}
    regions = {name: pk.regions for name, pk in packs.items()}
    return images, regions, bimg


def _flatten(params):
    out = {}
    for k, v in params.items():
        if k == "branches":
            for i, br in enumerate(v):
                for bn, bd in br.items():
                    for wn, wv in bd.items():
                        out[f"br{i}.{bn}.{wn}"] = wv
        else:
            out[k] = v
    return out


def _stem_images(xg):
    """xg: [BC, 25, 1000] gathered rows for one core.  Returns xa [125, 2000]
    (taps 0-4) and xb [50, 2000] (taps 5-6), with column order
    (chunk, b, t_local), chunk = 125 output positions."""
    xp = np.zeros((BC, N_ELECS, T + 6), np.float32)
    xp[:, :, 3 : 3 + T] = xg
    xa = np.empty((125, BC * L1), np.float32)
    xb = np.empty((50, BC * L1), np.float32)
    for j in range(7):
        arr = xp[:, :, j : j + 2 * L1 : 2]  # [BC, 25, 500]
        # -> [25, chunk, b, tl] -> [25, 2000]
        a4 = arr.transpose(1, 0, 2).reshape(N_ELECS, BC, 4, 125)
        # column order (chunk, t_local, b): batch-interleaved inner dim
        a4 = np.ascontiguousarray(a4.transpose(0, 2, 3, 1)).reshape(N_ELECS, BC * L1)
        if j < 5:
            xa[j * 25 : (j + 1) * 25] = a4
        else:
            xb[(j - 5) * 25 : (j - 4) * 25] = a4
    return xa, xb


# -------------------------------------------------------------- device side


def _build_program(wcols):
    import concourse.mybir as mybir
    from concourse import bacc
    from concourse.tile import TileContext

    F32 = mybir.dt.float32
    F32R = mybir.dt.bfloat16 if COMPUTE == "bf16" else mybir.dt.float32r
    AF = mybir.ActivationFunctionType
    ALU = mybir.AluOpType

    nc = bacc.Bacc("TRN2", target_bir_lowering=False, num_devices=NCORES)

    d_xa = nc.dram_tensor("xa", [125, BC * L1], F32R, kind="ExternalInput")
    d_xb = nc.dram_tensor("xb", [50, BC * L1], F32R, kind="ExternalInput")
    d_bs = nc.dram_tensor("bs", [128, 16], F32, kind="ExternalInput")
    d_w = {
        name: nc.dram_tensor(name, [128, wcols[name]], F32R, kind="ExternalInput")
        for name in ("w0", "w3", "w5", "w7")
    }
    d_y = nc.dram_tensor("y", [NCLS, BC], F32, kind="ExternalOutput")

    with TileContext(nc) as tc:
        with (
            tc.tile_pool(name="const", bufs=1) as cpool,
            tc.tile_pool(name="act", bufs=1) as apool,
            tc.tile_pool(name="work", bufs=2) as wpool,
            tc.tile_pool(name="pp", bufs=6, space="PSUM") as ppool,
            tc.tile_pool(name="ph", bufs=2, space="PSUM") as phpool,
        ):
            dma_engines = [nc.sync, nc.gpsimd]
            _ei = [0]

            def dma(out, in_):
                dma_engines[_ei[0] % len(dma_engines)].dma_start(out=out, in_=in_)
                _ei[0] += 1

            # preload the ACT function table while DMAs run
            dummy = wpool.tile([1, 1], F32, tag="dummy")
            nc.vector.memset(dummy[:], 0.0)
            nc.scalar.activation(dummy[:], dummy[:], AF.Relu)

            xa_t = cpool.tile([125, BC * L1], F32R, tag="xa")
            xb_t = cpool.tile([50, BC * L1], F32R, tag="xb")
            bs_t = cpool.tile([128, 16], F32, tag="bs")
            w_t = {}
            for img in ("w0", "w3", "w5", "w7"):
                w_t[img] = cpool.tile([128, wcols[img]], F32R, tag=img,
                                      name=f"{img}_t")
            stem_end = _REGIONS["w0"]["id64"][1]
            # b1-weight column boundary inside each branch image
            b1_end = {k: _REGIONS[f"w{k}"]["b2c1_p"][1] for k in KS}
            dma(xa_t[:, 0:252], d_xa[:, 0:252])
            dma(w_t["w0"][:, 0:stem_end], d_w["w0"][:, 0:stem_end])
            dma(xb_t[:, 0:252], d_xb[:, 0:252])
            dma(xa_t[:, 252:500], d_xa[:, 252:500])
            dma(xb_t[:, 252:500], d_xb[:, 252:500])
            dma(xa_t[:, 500:1000], d_xa[:, 500:1000])
            dma(bs_t[:], d_bs[:])
            dma(xa_t[:, 1000:1500], d_xa[:, 1000:1500])
            dma(xb_t[:, 500:1000], d_xb[:, 500:1000])
            dma(xa_t[:, 1500:2000], d_xa[:, 1500:2000])
            dma(xb_t[:, 1000:2000], d_xb[:, 1000:2000])
            for k in KS:
                dma(w_t[f"w{k}"][:, 0 : b1_end[k]], d_w[f"w{k}"][:, 0 : b1_end[k]])
            for k in KS:
                dma(w_t[f"w{k}"][:, b1_end[k] :], d_w[f"w{k}"][:, b1_end[k] :])
            dma(w_t["w0"][:, stem_end:], d_w["w0"][:, stem_end:])

            def wreg(img, rname, rows, j, cout):
                r0, c0, rr, cc = _REGIONS[img][rname]
                return w_t[img][0:rows, c0 + j * cout : c0 + (j + 1) * cout]

            def bias(col, rows=128):
                return bs_t[0:rows, col : col + 1]

            # ---- persistent tiles, batch-interleaved (col = BC*t + b).
            # d-tiles: rows 0:64 = tensor, rows 64:128 = tensor shifted by one
            # t (filled by an SBUF->SBUF DMA), enabling 128-deep tap pairs.
            h1 = apool.tile([64, BC * (L1 + 2)], F32R, tag="h1")
            h2d = apool.tile([128, BC * (L2 + 6)], F32R, tag="h2d")
            y1d = {k: apool.tile([128, BC * (L2 + 6)], F32R, tag=f"y1d_{k}", name=f"y1d_{k}") for k in KS}
            y2d = {k: apool.tile([128, BC * (L2 + 6)], F32R, tag=f"y2d_{k}", name=f"y2d_{k}") for k in KS}
            z1 = {k: apool.tile([128, BC * (L3 + 6)], F32R, tag=f"z1_{k}", name=f"z1_{k}") for k in KS}
            feats32 = apool.tile([128, 12], F32, tag="feats32")
            feats = apool.tile([128, 12], F32R, tag="feats")

            def pad_memset(tile, parts, lp, interior_l, lpad):
                u32 = mybir.dt.uint32
                if lpad:
                    nc.vector.memset(tile[0:parts, 0 : BC * lpad].bitcast(u32), 0)
                rs = lpad + interior_l
                if rs < lp:
                    nc.vector.memset(tile[0:parts, BC * rs : BC * lp].bitcast(u32), 0)

            pad_memset(h1, 64, L1 + 2, L1, 1)
            pad_memset(h2d, 64, L2 + 6, L2, 3)
            for k in KS:
                pad_memset(y1d[k], 64, L2 + 6, L2, 3)
                pad_memset(y2d[k], 64, L2 + 6, L2, 3)
                pad_memset(z1[k], 128, L3 + 6, L3, 3)

            def shift_fill(dt_, lp):
                # bottom[t] = top[t+1]; last bottom column stays unread
                dma(dt_[64:128, 0 : BC * (lp - 1)], dt_[0:64, BC * 1 : BC * lp])

            # ---- stem (first chunks smaller so compute starts earlier)
            for t0c, cn in ((0, 63), (63, 62), (125, 125), (250, 125), (375, 125)):
                pt = ppool.tile([128, 500], F32, tag="pt")
                cs = slice(BC * t0c, BC * (t0c + cn))
                nc.tensor.matmul(pt[0:64, 0 : BC * cn],
                                 lhsT=wreg("w0", "stemA", 125, 0, 64),
                                 rhs=xa_t[:, cs], start=True, stop=False)
                nc.tensor.matmul(pt[0:64, 0 : BC * cn],
                                 lhsT=wreg("w0", "stemB", 50, 0, 64),
                                 rhs=xb_t[:, cs], start=False, stop=True)
                nc.scalar.activation(
                    h1[0:64, BC * (1 + t0c) : BC * (1 + t0c + cn)],
                    pt[0:64, 0 : BC * cn], AF.Relu, bias=bias(0, 64),
                )

            # ---- maxpool 3 s2 p1 into h2d top
            h1w = h1.rearrange("c (t b) -> c t b", b=BC)
            pm = wpool.tile([64, BC * 250], F32R, tag="pm")
            for t0, w in ((0, 128), (128, 122)):
                a = h1w[0:64, 2 * t0 + 0 : 2 * (t0 + w) + 0 : 2, :]
                bb = h1w[0:64, 2 * t0 + 1 : 2 * (t0 + w) + 1 : 2, :]
                cc = h1w[0:64, 2 * t0 + 2 : 2 * (t0 + w) + 2 : 2, :]
                pmv = pm[:, BC * t0 : BC * (t0 + w)].rearrange(
                    "c (t b) -> c t b", b=BC)
                h2v = h2d[0:64, BC * (3 + t0) : BC * (3 + t0 + w)].rearrange(
                    "c (t b) -> c t b", b=BC)
                nc.vector.tensor_tensor(pmv, a, bb, ALU.max)
                nc.vector.tensor_tensor(h2v, pmv, cc, ALU.max)
                # shift as soon as this half (plus pads) is in place
                if t0 == 0:
                    dma(h2d[64:128, 0 : BC * 130], h2d[0:64, BC * 1 : BC * 131])
                else:
                    dma(h2d[64:128, BC * 130 : BC * 255],
                        h2d[0:64, BC * 131 : BC * 256])

            # ---- paired-tap conv over a d-tile (stride 1 contiguous slices)
            def conv_pair(img, cname, dt_, cout, k, Lout, out_t=None, out_off=0,
                          bias_col=None, epilogue=None, extra=None, stride=1):
                nch = (BC * Lout + 499) // 500
                cn = Lout // nch
                dt3 = dt_.rearrange("c (t b) -> c t b", b=BC)
                for c in range(nch):
                    pt = ppool.tile([128, 500], F32, tag="pt")
                    po = pt[0:cout, 0 : BC * cn]
                    groups = _groups(k)
                    # solos first: they read only the unshifted top half, so
                    # they can start before the shift DMA lands
                    order = [g for g in groups if not g[1]] + [g for g in groups if g[1]]
                    pidx = {}
                    np_, ns_ = 0, 0
                    for o, ispair in groups:
                        if ispair:
                            pidx[o] = np_; np_ += 1
                        else:
                            pidx[o] = ns_; ns_ += 1
                    for gi, (o, ispair) in enumerate(order):
                        lastg = gi == len(order) - 1
                        if ispair:
                            lhsT = wreg(img, f"{cname}_p", 128, pidx[o], cout)
                            rows = 128
                        else:
                            lhsT = wreg(img, f"{cname}_s", 64, pidx[o], cout)
                            rows = 64
                        if stride == 1:
                            rhs = dt_[0:rows, BC * (o + c * cn) : BC * (o + c * cn + cn)]
                        else:
                            rhs = dt3[0:rows, o : o + 2 * cn : 2, :]
                        nc.tensor.matmul(po, lhsT=lhsT, rhs=rhs,
                                         start=(gi == 0),
                                         stop=(lastg and extra is None))
                    if extra is not None:
                        extra(po, c, cn)
                    if epilogue == "act":
                        nc.scalar.activation(
                            out_t[0:cout, BC * (out_off + c * cn) : BC * (out_off + (c + 1) * cn)],
                            po, AF.Relu, bias=bias(bias_col, cout),
                        )
                    else:
                        epilogue(pt, c, cn)

            # ---- wave-interleaved branch stages
            for i, k in enumerate(KS):
                conv_pair(f"w{k}", "b1c1", h2d, 64, k, L2,
                          out_t=y1d[k], out_off=3, bias_col=1 + 4 * i,
                          epilogue="act")
            for k in KS:
                shift_fill(y1d[k], L2 + 6)

            for i, k in enumerate(KS):
                def sc_extra(po, c, cn):
                    nc.tensor.matmul(
                        po, lhsT=wreg("w0", "id64", 64, 0, 64),
                        rhs=h2d[0:64, BC * (3 + c * cn) : BC * (3 + (c + 1) * cn)],
                        start=False, stop=True,
                    )
                conv_pair(f"w{k}", "b1c2", y1d[k], 64, k, L2,
                          out_t=y2d[k], out_off=3, bias_col=2 + 4 * i,
                          epilogue="act", extra=sc_extra)
            for k in KS:
                shift_fill(y2d[k], L2 + 6)

            def b2c1_stage(i, k):
                conv_pair(f"w{k}", "b2c1", y2d[k], 128, k, L3,
                          out_t=z1[k], out_off=3, bias_col=3 + 4 * i,
                          epilogue="act", stride=2)

            def b2c2_stage(i, k):
                wk = f"w{k}"
                y2d3 = y2d[k].rearrange("c (t b) -> c t b", b=BC)

                def ds_extra(po, c, cn, _k=k, _y=y2d3):
                    nc.tensor.matmul(
                        po, lhsT=wreg(f"w{_k}", "down", 64, 0, 128),
                        rhs=_y[0:64, 3 : 253 : 2, :],
                        start=False, stop=True,
                    )

                def pool_epilogue(pt, c, cn, _i=i):
                    rt = wpool.tile([128, BC * L3], F32R, tag="rt")
                    nc.vector.tensor_scalar(
                        rt[:], pt[0:128, 0 : BC * L3], bias(4 + 4 * _i), 0.0,
                        ALU.add, ALU.max,
                    )
                    with nc.allow_low_precision(reason="bf16 feature sums"):
                        nc.vector.tensor_reduce(
                            feats[:, _i * 4 : (_i + 1) * 4],
                            rt.rearrange("c (t b) -> c b t", b=BC),
                            mybir.AxisListType.X, ALU.add,
                        )

                # b2c2: full-128 contraction, one tap per matmul
                p = (k - 1) // 2
                pt = ppool.tile([128, 500], F32, tag="pt")
                po = pt[0:128, 0 : BC * L3]
                for j in range(k):
                    o = (3 - p) + j
                    rhs = z1[k][0:128, BC * o : BC * (o + L3)]
                    nc.tensor.matmul(po, lhsT=wreg(wk, "b2c2", 128, j, 128),
                                     rhs=rhs, start=(j == 0), stop=False)
                ds_extra(po, 0, L3)
                pool_epilogue(pt, 0, L3)

            # interleave so each b2c2 waits behind other ready work
            b2c1_stage(0, 3)
            b2c1_stage(1, 5)
            b2c2_stage(0, 3)
            b2c1_stage(2, 7)
            b2c2_stage(1, 5)
            b2c2_stage(2, 7)

            # ---- head
            pe = phpool.tile([128, 8], F32, tag="ph")
            for br in range(3):
                nc.tensor.matmul(
                    pe[:, 0:4], lhsT=wreg("w0", "fc", 128, br, 128),
                    rhs=feats[:, br * 4 : (br + 1) * 4],
                    start=(br == 0), stop=(br == 2),
                )
            embt = wpool.tile([128, 4], F32R, tag="embt")
            nc.scalar.activation(embt[:], pe[:, 0:4], AF.Identity, bias=bias(13))

            ph2 = phpool.tile([128, 8], F32, tag="ph")
            nc.tensor.matmul(ph2[:, 0:4], lhsT=wreg("w0", "cl1", 128, 0, 128),
                             rhs=embt[:], start=True, stop=True)
            hct = wpool.tile([128, 4], F32R, tag="hct")
            nc.scalar.activation(hct[:], ph2[:, 0:4], AF.Relu, bias=bias(14))

            ph3 = phpool.tile([128, 8], F32, tag="ph")
            nc.tensor.matmul(ph3[0:NCLS, 0:4], lhsT=wreg("w0", "cl2", 128, 0, NCLS),
                             rhs=hct[:], start=True, stop=True)
            outt = wpool.tile([NCLS, 4], F32, tag="outt")
            nc.scalar.activation(outt[:], ph3[0:NCLS, 0:4], AF.Identity,
                                 bias=bias(15, NCLS))
            nc.sync.dma_start(out=d_y[:], in_=outt[:])

    nc.compile()
    return nc


_REGIONS = None


def _prepare(x, padding_mask, params):
    global _REGIONS
    x = np.asarray(x, np.float32)
    padding_mask = np.asarray(padding_mask, bool)
    sel, trial = _sample_indices(padding_mask)
    images, regions, bimg = _pack_weights(params)
    cimages = {k: _to_compute(v) for k, v in images.items()}
    _REGIONS = regions

    in_maps = []
    for c in range(NCORES):
        bs = slice(c * BC, (c + 1) * BC)
        xs = x[bs]
        xg = xs[np.arange(BC)[:, None], sel[bs], trial[bs]]  # [BC, 25, 1000]
        xa, xb = _stem_images(xg)
        im = {"xa": _to_compute(xa), "xb": _to_compute(xb), "bs": bimg}
        im.update(cimages)
        in_maps.append(im)
    wcols = {name: arr.shape[1] for name, arr in images.items()}
    return in_maps, wcols


def kernel(x, padding_mask, params):
    global _PROG, LAST_RESULTS
    from concourse.bass_utils import run_bass_kernel_spmd

    in_maps, wcols = _prepare(x, padding_mask, params)
    if _PROG is None:
        _PROG = _build_program(wcols)

    trace = os.environ.get("BASS_KERNEL_TRACE", "0") == "1"
    res = run_bass_kernel_spmd(
        _PROG, in_maps, core_ids=list(range(NCORES)), trace=trace
    )
    LAST_RESULTS = res
    out = np.empty((B, NCLS), np.float32)
    for c in range(NCORES):
        out[c * BC : (c + 1) * BC] = res.results[c]["y"].T
    return out
